# revision 39
# baseline (speedup 1.0000x reference)
"""Single-head causal self-attention on 8 Trainium2 NeuronCores.

Problem: x[8, 2048, 1024], Wq/Wk/Wv[1024, 64] ->
  out[b] = softmax(causal((x[b]@Wq) @ (x[b]@Wk)^T / 8)) @ (x[b]@Wv)

Sharding: data-parallel over batch B=8, one batch element per core; weights
replicated. All device traffic is bf16 (PSUM accumulation stays fp32).

Per-core scheme:
  - [q^T;k^T] = Wqk^T @ x^T   (PE; x^T streamed per t-chunk)
  - V[t-tile, 64] natural layout directly: x^T-tile stationary, Wv moving
    (64-col matmuls) -> no V transpose; col 64 of V_aug = ones
  - S^T[j-tile, q-chunk] = (k^T tile)^T @ q^T, causal blocks only; two
    full-width blocks share one [128,1024] PSUM tile (2 banks) so a single
    exp instruction covers both (ACT per-instruction overhead is 185ns)
  - P^T = exp(S^T / 8)  (ACT, PSUM->SBUF bf16; no max-subtraction: scaled
    scores are ~N(0,1) so exp is well-conditioned)
  - diagonal 128x128 sub-blocks multiplied by a 0/1 triangle mask (DVE)
  - AV q-major: per q-tile, accumulate sum_j P^T-slice.T @ V_aug[j]
    (65-col matmuls, natural output; ones column = softmax denominator),
    then normalize rows by reciprocal of col 64 (DVE) straight from PSUM

Scheduling: ACT's ~21us of exp work is the critical path.  The emission
order forms one global pipeline: projections for chunk c+1 are spliced a
few score-blocks before the end of chunk c (hiding the proj->copy->S
latency), AV passes for chunk c run early in chunk c+1's stream, and the
first chunks are narrow so exp starts as early as possible.
"""

import numpy as np

import concourse.bass as bass
import concourse.mybir as mybir
import concourse.tile as tile
from concourse import bacc
from concourse.bass_utils import run_bass_kernel_spmd
from concourse.masks import make_upper_triangular

N_CORES = 8
B, T, C, D = 8, 2048, 1024, 64
CT = C // 128          # 8 contraction tiles
NT = T // 128          # 16 row tiles
SCALE = float(1.0 / np.sqrt(D))

FP = mybir.dt.float32
BF = mybir.dt.bfloat16
XDT = mybir.dt.bfloat16  # fp8 fails the 2e-2 gate (scores + v for low-n_eff rows)

CHUNKS = [(0, 256), (256, 256), (512, 256), (768, 256), (1024, 512), (1536, 512)]
XSEGS = [(512, 768), (768, 1024), (1024, 1536), (1536, 2048)]  # after x0,wqk,x1,wv
WARMUP = 26


def build_nc():
    nc = bacc.Bacc("TRN2", target_bir_lowering=False)
    # x^T in t-tile-major layout [p, t-tile, ct, 128] so chunk DMAs are
    # contiguous per partition (>=1KB descriptors, no sub-512B penalty)
    xT_h = nc.dram_tensor("xT", [128, NT, CT, 128], XDT, kind="ExternalInput")
    wqk_h = nc.dram_tensor("wqk", [128, CT, 128], XDT, kind="ExternalInput")
    wv_h = nc.dram_tensor("wv", [128, CT, D], XDT, kind="ExternalInput")
    y_h = nc.dram_tensor("y", [128, NT, D], BF, kind="ExternalOutput")

    with tile.TileContext(nc) as tc:
        with (
            tc.tile_pool(name="const", bufs=1) as const,
            tc.tile_pool(name="pt", bufs=14) as ptp,
            tc.tile_pool(name="sc", bufs=4) as scp,
            tc.tile_pool(name="ps_w", bufs=2, space="PSUM") as ps_w,
            tc.tile_pool(name="ps_s", bufs=2, space="PSUM") as ps_s,
            tc.tile_pool(name="ps_o", bufs=2, space="PSUM") as ps_o,
        ):
            # ---- constants ----
            tri = const.tile([128, 128], BF, tag="tri")  # tri[p,f]=1.0 iff f>=p
            make_upper_triangular(nc, tri, val=1.0, diag=True)

            wqk_sb = const.tile([128, CT, 128], XDT, tag="wqk")
            wv_sb = const.tile([128, CT, D], XDT, tag="wv")
            xT_sb = const.tile([128, NT, CT, 128], XDT, tag="xT")
            # fine x segments up front (early projections), coarse later;
            # weights interleaved so x(0:256) lands first, wv by first AV
            def xseg(a, b):
                nc.sync.dma_start(
                    out=xT_sb[:, a // 128 : b // 128, :, :],
                    in_=xT_h[:, a // 128 : b // 128, :, :],
                )
            xseg(0, 256)
            nc.sync.dma_start(out=wqk_sb, in_=wqk_h[:, :, :])
            xseg(256, 512)
            nc.sync.dma_start(out=wv_sb, in_=wv_h[:, :, :])
            for a, b in XSEGS:
                xseg(a, b)

            # PE clock warmup: dummy matmuls while the first DMAs land (the
            # p-state ramp needs ~3us of continuous PE activity)
            if WARMUP:
                wu = ps_w.tile([128, 512], FP, tag="w", name="wu")
                for i in range(WARMUP):
                    nc.tensor.matmul(
                        wu[:, 0:128], tri, tri, start=True, stop=True,
                        skip_group_check=True,
                    )

            qT = const.tile([64, T], BF, tag="qT")
            kT = const.tile([64, T], BF, tag="kT")
            V = const.tile([128, NT, D + 1], BF, tag="V")  # col D = ones
            nc.gpsimd.memset(V[:, :, D], 1.0)
            out_sb = const.tile([128, NT, D], BF, tag="out")

            # ---- emission helpers ----
            def proj_qk_steps(a, b, nsteps=4):
                """Emit the qk projection for t-cols [a,b) as `nsteps` thunks
                so the 8 accumulation matmuls can interleave with S blocks
                in the PE FIFO instead of forming a 1.7us monolith."""
                w = b - a
                state = {}

                def step(c0, c1, last):
                    if not state:
                        state["t"] = ps_w.tile(
                            [128, 512], FP, tag="w", name=f"pqk_{a}"
                        )
                    p_qk = state["t"]
                    for ct in range(c0, c1):
                        nc.tensor.matmul(
                            p_qk[:, 0:w],
                            wqk_sb[:, ct, :],
                            xT_sb[:, a // 128 : b // 128, ct, :],
                            start=(ct == 0),
                            stop=(ct == CT - 1),
                        )
                    if last:
                        nc.vector.tensor_copy(qT[:, a:b], p_qk[0:64, 0:w])
                        nc.vector.tensor_copy(kT[:, a:b], p_qk[64:128, 0:w])

                bounds = [CT * i // nsteps for i in range(nsteps + 1)]
                return [
                    (lambda c0=bounds[i], c1=bounds[i + 1], last=(i == nsteps - 1):
                     step(c0, c1, last))
                    for i in range(nsteps)
                ]

            def proj_qk(a, b):
                for f in proj_qk_steps(a, b, 1):
                    f()

            def proj_v(jt):
                p_v = ps_w.tile([128, 512], FP, tag="w", name=f"pv_{jt}")
                for ct in range(CT):
                    nc.tensor.matmul(
                        p_v[:, 0:D],
                        xT_sb[:, jt, ct, :],
                        wv_sb[:, ct, :],
                        start=(ct == 0),
                        stop=(ct == CT - 1),
                    )
                nc.vector.tensor_copy(V[:, jt, 0:D], p_v[:, 0:D])  # GPSIMD can't read PSUM

            # pt_view[(ci, jt)] = (tile, col offset) for AV consumption
            pt_view = {}

            def s_single(ci, jt):
                qs, qw = CHUNKS[ci]
                lo = max(jt * 128 - qs, 0)
                p_s = ps_s.tile([128, 1024], FP, tag="s", name=f"s_{ci}_{jt}")
                nc.tensor.matmul(
                    p_s[:, lo:qw],
                    kT[:, jt * 128 : (jt + 1) * 128],
                    qT[:, qs + lo : qs + qw],
                    start=True,
                    stop=True,
                )
                pt = ptp.tile([128, 1024], BF, tag="pt", name=f"pt_{ci}_{jt}")
                nc.scalar.activation(
                    pt[:, lo:qw],
                    p_s[:, lo:qw],
                    mybir.ActivationFunctionType.Exp,
                    scale=SCALE,
                )
                d0 = jt * 128 - qs
                if d0 >= 0:
                    nc.vector.tensor_mul(
                        pt[:, d0 : d0 + 128], pt[:, d0 : d0 + 128], tri
                    )
                pt_view[(ci, jt)] = (pt, 0)

            def s_pair(ci, jt):
                # two full-width (512) non-diagonal blocks, one exp
                qs, qw = CHUNKS[ci]
                p_s = ps_s.tile([128, 1024], FP, tag="s", name=f"s_{ci}_{jt}")
                for h, j in enumerate((jt, jt + 1)):
                    nc.tensor.matmul(
                        p_s[:, h * 512 : h * 512 + qw],
                        kT[:, j * 128 : (j + 1) * 128],
                        qT[:, qs : qs + qw],
                        start=True,
                        stop=True,
                    )
                pt = ptp.tile([128, 1024], BF, tag="pt", name=f"pt_{ci}_{jt}")
                nc.scalar.activation(
                    pt[:, 0:1024],
                    p_s[:, 0:1024],
                    mybir.ActivationFunctionType.Exp,
                    scale=SCALE,
                )
                pt_view[(ci, jt)] = (pt, 0)
                pt_view[(ci, jt + 1)] = (pt, 512)

            def av_pass(ci, qi):
                qs, qw = CHUNKS[ci]
                qt = qs // 128 + qi
                ot = ps_o.tile([128, D + 1], FP, tag="o", name=f"o_{qt}")
                for jt in range(qt + 1):
                    pt, off = pt_view[(ci, jt)]
                    lo = max(jt * 128 - qs, 0)
                    assert qi * 128 >= lo
                    nc.tensor.matmul(
                        ot,
                        pt[:, off + qi * 128 : off + (qi + 1) * 128],
                        V[:, jt, :],
                        start=(jt == 0),
                        stop=(jt == qt),
                    )
                s2 = scp.tile([128, 2], FP, tag="s2", name=f"s2_{qt}")
                nc.vector.tensor_copy(s2[:, 0:1], ot[:, D : D + 1])
                nc.vector.reciprocal(s2[:, 1:2], s2[:, 0:1])
                nc.vector.tensor_scalar_mul(
                    out_sb[:, qt, :], ot[:, 0:D], s2[:, 1:2]
                )

            def av_chunk(ci, qi=None):
                qs, qw = CHUNKS[ci]
                rng = range(qw // 128) if qi is None else [qi]
                for q in rng:
                    av_pass(ci, q)
                if qi is not None and qi != qw // 128 - 1:
                    return
                hi = (qs + qw) // 128
                if hi % 4 == 0:
                    nc.sync.dma_start(
                        out=y_h[:, hi - 4 : hi, :], in_=out_sb[:, hi - 4 : hi, :]
                    )

            def chunk_items(ci):
                qs, qw = CHUNKS[ci]
                n_jt = (qs + qw) // 128
                nd = qs // 128  # non-diagonal j-tiles
                items = []
                if qw == 512:
                    for j in range(0, nd, 2):
                        items.append(lambda j=j: s_pair(ci, j))
                    start_diag = nd
                else:
                    start_diag = 0
                for j in range(start_diag, n_jt):
                    items.append(lambda j=j: s_single(ci, j))
                return items

            # ---- global emission ----
            # chunk items with splices: AV of previous chunk early, proj of
            # next chunk's qk + this chunk's v late (hide proj latency).
            # Processing order pulls the first 512-chunk before (256,256).
            # Background-work queue: next chunk's projection steps, this
            # chunk's v projections, and the previous chunk's AV passes are
            # dribbled out between S items so the PE FIFO never carries a
            # long burst that would starve ACT of score blocks.
            proj_qk(0, 256)
            for ci in range(len(CHUNKS)):
                qs, qw = CHUNKS[ci]
                items = chunk_items(ci)
                n = len(items)
                bg = []
                if ci + 1 < len(CHUNKS):
                    nqs, nqw = CHUNKS[ci + 1]
                    bg += proj_qk_steps(nqs, nqs + nqw, 2 if nqw <= 256 else 4)
                for jj in range(qs // 128, (qs + qw) // 128):
                    bg.append(lambda j=jj: proj_v(j))
                if ci > 0:
                    for k in range(CHUNKS[ci - 1][1] // 128):
                        bg.append(lambda k=k: av_chunk(ci - 1, k))
                for idx, it in enumerate(items):
                    it()
                    rem = n - 1 - idx
                    take = len(bg) if rem == 0 else -(-len(bg) // max(rem, 1))
                    for _ in range(min(take, len(bg))):
                        bg.pop(0)()
            av_chunk(len(CHUNKS) - 1)

    nc.finalize()
    return nc


_NC_CACHE = None
LAST_RESULTS = None


def kernel(x, Wq, Wk, Wv, trace=False, **run_kwargs):
    global _NC_CACHE, LAST_RESULTS
    import ml_dtypes

    xdt = ml_dtypes.bfloat16
    x = np.asarray(x, dtype=np.float32)
    wqk = np.concatenate(
        [np.asarray(Wq, np.float32), np.asarray(Wk, np.float32)], axis=1
    ).astype(xdt)
    wqk_p = np.ascontiguousarray(wqk.reshape(CT, 128, 128).swapaxes(0, 1))
    wv_p = np.ascontiguousarray(
        np.asarray(Wv, np.float32).astype(xdt).reshape(CT, 128, D).swapaxes(0, 1)
    )

    if _NC_CACHE is None:
        _NC_CACHE = build_nc()
    nc = _NC_CACHE

    in_maps = []
    for b in range(N_CORES):
        xT = x[b].T.astype(xdt)  # [C, T]
        # [C, T] -> [p, t-tile, ct, 128]
        xT_p = np.ascontiguousarray(
            xT.reshape(CT, 128, NT, 128).transpose(1, 2, 0, 3)
        )
        in_maps.append({"xT": xT_p, "wqk": wqk_p, "wv": wv_p})

    res = run_bass_kernel_spmd(
        nc, in_maps, core_ids=list(range(N_CORES)), trace=trace, **run_kwargs
    )
    LAST_RESULTS = res
    out = np.stack(
        [
            np.asarray(res.results[b]["y"])
            .swapaxes(0, 1)
            .reshape(T, D)
            .astype(np.float32)
            for b in range(N_CORES)
        ],
        axis=0,
    )
    return out


if __name__ == "__main__":
    rng = np.random.default_rng(0)
    x = rng.standard_normal((B, T, C), dtype=np.float32)
    s = 1.0 / np.sqrt(C)
    Wq = rng.standard_normal((C, D), dtype=np.float32) * s
    Wk = rng.standard_normal((C, D), dtype=np.float32) * s
    Wv = rng.standard_normal((C, D), dtype=np.float32) * s
    out = kernel(x, Wq, Wk, Wv)
    print("out", out.shape, out.dtype, float(np.abs(out).max()))


# revision 45
# speedup vs baseline: 1.0113x; 1.0113x over previous
"""Single-head causal self-attention on 8 Trainium2 NeuronCores.

Problem: x[8, 2048, 1024], Wq/Wk/Wv[1024, 64] ->
  out[b] = softmax(causal((x[b]@Wq) @ (x[b]@Wk)^T / 8)) @ (x[b]@Wv)

Sharding: data-parallel over batch B=8, one batch element per core; weights
replicated. All device traffic is bf16 (PSUM accumulation stays fp32).

Per-core scheme:
  - [q^T;k^T] = Wqk^T @ x^T   (PE; x^T streamed per t-chunk)
  - V[t-tile, 64] natural layout directly: x^T-tile stationary, Wv moving
    (64-col matmuls) -> no V transpose; col 64 of V_aug = ones
  - S^T[j-tile, q-chunk] = (k^T tile)^T @ q^T, causal blocks only; two
    full-width blocks share one [128,1024] PSUM tile (2 banks) so a single
    exp instruction covers both (ACT per-instruction overhead is 185ns)
  - P^T = exp(S^T / 8)  (ACT, PSUM->SBUF bf16; no max-subtraction: scaled
    scores are ~N(0,1) so exp is well-conditioned)
  - diagonal 128x128 sub-blocks multiplied by a 0/1 triangle mask (DVE)
  - AV q-major: per q-tile, accumulate sum_j P^T-slice.T @ V_aug[j]
    (65-col matmuls, natural output; ones column = softmax denominator),
    then normalize rows by reciprocal of col 64 (DVE) straight from PSUM

Scheduling: ACT's ~21us of exp work is the critical path.  The emission
order forms one global pipeline: projections for chunk c+1 are spliced a
few score-blocks before the end of chunk c (hiding the proj->copy->S
latency), AV passes for chunk c run early in chunk c+1's stream, and the
first chunks are narrow so exp starts as early as possible.
"""

import numpy as np

import concourse.bass as bass
import concourse.mybir as mybir
import concourse.tile as tile
from concourse import bacc
from concourse.bass_utils import run_bass_kernel_spmd
from concourse.masks import make_upper_triangular

N_CORES = 8
B, T, C, D = 8, 2048, 1024, 64
CT = C // 128          # 8 contraction tiles
NT = T // 128          # 16 row tiles
SCALE = float(1.0 / np.sqrt(D))

FP = mybir.dt.float32
BF = mybir.dt.bfloat16
XDT = mybir.dt.bfloat16  # fp8 fails the 2e-2 gate (scores + v for low-n_eff rows)

CHUNKS = [(0, 256), (256, 256), (512, 256), (768, 256), (1024, 512), (1536, 512)]
XSEGS = [(512, 768), (768, 1024), (1024, 1536), (1536, 2048)]  # after x0,wqk,x1,wv
WARMUP = 26


def build_nc():
    nc = bacc.Bacc("TRN2", target_bir_lowering=False)
    # x^T in t-tile-major layout [p, t-tile, ct, 128] so chunk DMAs are
    # contiguous per partition (>=1KB descriptors, no sub-512B penalty)
    xT_h = nc.dram_tensor("xT", [128, NT, CT, 128], XDT, kind="ExternalInput")
    wqk_h = nc.dram_tensor("wqk", [128, CT, 128], XDT, kind="ExternalInput")
    wv_h = nc.dram_tensor("wv", [128, CT, D], XDT, kind="ExternalInput")
    y_h = nc.dram_tensor("y", [128, NT, D], BF, kind="ExternalOutput")

    with tile.TileContext(nc) as tc:
        with (
            tc.tile_pool(name="const", bufs=1) as const,
            tc.tile_pool(name="pt", bufs=17) as ptp,
            tc.tile_pool(name="sc", bufs=8) as scp,
            tc.tile_pool(name="ps_w", bufs=2, space="PSUM") as ps_w,
            tc.tile_pool(name="ps_s", bufs=2, space="PSUM") as ps_s,
            tc.tile_pool(name="ps_o", bufs=2, space="PSUM") as ps_o,
        ):
            # ---- constants ----
            tri = const.tile([128, 128], BF, tag="tri")  # tri[p,f]=1.0 iff f>=p
            make_upper_triangular(nc, tri, val=1.0, diag=True)

            wqk_sb = const.tile([128, CT, 128], XDT, tag="wqk")
            wv_sb = const.tile([128, CT, D], XDT, tag="wv")
            xT_sb = const.tile([128, NT, CT, 128], XDT, tag="xT")
            # fine x segments up front (early projections), coarse later;
            # weights interleaved so x(0:256) lands first, wv by first AV
            def xseg(a, b):
                nc.sync.dma_start(
                    out=xT_sb[:, a // 128 : b // 128, :, :],
                    in_=xT_h[:, a // 128 : b // 128, :, :],
                )
            xseg(0, 256)
            nc.sync.dma_start(out=wqk_sb, in_=wqk_h[:, :, :])
            xseg(256, 512)
            xseg(*XSEGS[0])
            nc.sync.dma_start(out=wv_sb, in_=wv_h[:, :, :])
            for a, b in XSEGS[1:]:
                xseg(a, b)

            # PE clock warmup: dummy matmuls while the first DMAs land (the
            # p-state ramp needs ~3us of continuous PE activity)
            if WARMUP:
                wu = ps_w.tile([128, 512], FP, tag="w", name="wu")
                for i in range(WARMUP):
                    nc.tensor.matmul(
                        wu[:, 0:128], tri, tri, start=True, stop=True,
                        skip_group_check=True,
                    )

            qT = const.tile([64, T], BF, tag="qT")
            kT = const.tile([64, T], BF, tag="kT")
            V = const.tile([128, NT, D + 1], BF, tag="V")  # col D = ones
            nc.gpsimd.memset(V[:, :, D], 1.0)
            out_sb = const.tile([128, NT, D], BF, tag="out")

            # ---- emission helpers ----
            def proj_qk_steps(a, b, nsteps=4):
                """Emit the qk projection for t-cols [a,b) as `nsteps` thunks
                so the 8 accumulation matmuls can interleave with S blocks
                in the PE FIFO instead of forming a 1.7us monolith."""
                w = b - a
                state = {}

                def step(c0, c1, last):
                    if not state:
                        state["t"] = ps_w.tile(
                            [128, 512], FP, tag="w", name=f"pqk_{a}"
                        )
                    p_qk = state["t"]
                    for ct in range(c0, c1):
                        nc.tensor.matmul(
                            p_qk[:, 0:w],
                            wqk_sb[:, ct, :],
                            xT_sb[:, a // 128 : b // 128, ct, :],
                            start=(ct == 0),
                            stop=(ct == CT - 1),
                        )
                    if last:
                        if a < 512:  # ACT is idle this early; halve the chain
                            nc.scalar.copy(qT[:, a:b], p_qk[0:64, 0:w])
                        else:
                            nc.vector.tensor_copy(qT[:, a:b], p_qk[0:64, 0:w])
                        nc.vector.tensor_copy(kT[:, a:b], p_qk[64:128, 0:w])

                bounds = [CT * i // nsteps for i in range(nsteps + 1)]
                return [
                    (lambda c0=bounds[i], c1=bounds[i + 1], last=(i == nsteps - 1):
                     step(c0, c1, last))
                    for i in range(nsteps)
                ]

            def proj_qk(a, b):
                for f in proj_qk_steps(a, b, 1):
                    f()

            def proj_v(jt):
                p_v = ps_w.tile([128, 512], FP, tag="w", name=f"pv_{jt}")
                for ct in range(CT):
                    nc.tensor.matmul(
                        p_v[:, 0:D],
                        xT_sb[:, jt, ct, :],
                        wv_sb[:, ct, :],
                        start=(ct == 0),
                        stop=(ct == CT - 1),
                    )
                nc.vector.tensor_copy(V[:, jt, 0:D], p_v[:, 0:D])  # GPSIMD can't read PSUM

            # pt_view[(ci, jt)] = (tile, col offset) for AV consumption
            pt_view = {}

            def s_single(ci, jt):
                qs, qw = CHUNKS[ci]
                lo = max(jt * 128 - qs, 0)
                p_s = ps_s.tile([128, 1024], FP, tag="s", name=f"s_{ci}_{jt}")
                nc.tensor.matmul(
                    p_s[:, lo:qw],
                    kT[:, jt * 128 : (jt + 1) * 128],
                    qT[:, qs + lo : qs + qw],
                    start=True,
                    stop=True,
                )
                pt = ptp.tile([128, 1024], BF, tag="pt", name=f"pt_{ci}_{jt}")
                nc.scalar.activation(
                    pt[:, lo:qw],
                    p_s[:, lo:qw],
                    mybir.ActivationFunctionType.Exp,
                    scale=SCALE,
                )
                d0 = jt * 128 - qs
                if d0 >= 0:
                    nc.vector.tensor_mul(
                        pt[:, d0 : d0 + 128], pt[:, d0 : d0 + 128], tri
                    )
                pt_view[(ci, jt)] = (pt, 0)

            def s_pair(ci, jt):
                # two full-width (512) non-diagonal blocks, one exp
                qs, qw = CHUNKS[ci]
                p_s = ps_s.tile([128, 1024], FP, tag="s", name=f"s_{ci}_{jt}")
                for h, j in enumerate((jt, jt + 1)):
                    nc.tensor.matmul(
                        p_s[:, h * 512 : h * 512 + qw],
                        kT[:, j * 128 : (j + 1) * 128],
                        qT[:, qs : qs + qw],
                        start=True,
                        stop=True,
                    )
                pt = ptp.tile([128, 1024], BF, tag="pt", name=f"pt_{ci}_{jt}")
                nc.scalar.activation(
                    pt[:, 0:1024],
                    p_s[:, 0:1024],
                    mybir.ActivationFunctionType.Exp,
                    scale=SCALE,
                )
                pt_view[(ci, jt)] = (pt, 0)
                pt_view[(ci, jt + 1)] = (pt, 512)

            def av_pass(ci, qi):
                qs, qw = CHUNKS[ci]
                qt = qs // 128 + qi
                ot = ps_o.tile([128, D + 1], FP, tag="o", name=f"o_{qt}")
                for jt in range(qt + 1):
                    pt, off = pt_view[(ci, jt)]
                    lo = max(jt * 128 - qs, 0)
                    assert qi * 128 >= lo
                    nc.tensor.matmul(
                        ot,
                        pt[:, off + qi * 128 : off + (qi + 1) * 128],
                        V[:, jt, :],
                        start=(jt == 0),
                        stop=(jt == qt),
                    )
                s2 = scp.tile([128, 2], FP, tag="s2", name=f"s2_{qt}")
                nc.vector.tensor_copy(s2[:, 0:1], ot[:, D : D + 1])
                nc.vector.reciprocal(s2[:, 1:2], s2[:, 0:1])
                nc.vector.tensor_scalar_mul(
                    out_sb[:, qt, :], ot[:, 0:D], s2[:, 1:2]
                )

            def av_chunk(ci, qi=None):
                qs, qw = CHUNKS[ci]
                rng = range(qw // 128) if qi is None else [qi]
                for q in rng:
                    av_pass(ci, q)
                if qi is not None and qi != qw // 128 - 1:
                    return
                hi = (qs + qw) // 128
                if hi % 4 == 0:
                    nc.sync.dma_start(
                        out=y_h[:, hi - 4 : hi, :], in_=out_sb[:, hi - 4 : hi, :]
                    )

            def chunk_items(ci):
                qs, qw = CHUNKS[ci]
                n_jt = (qs + qw) // 128
                nd = qs // 128  # non-diagonal j-tiles
                items = []
                if qw == 512:
                    for j in range(0, nd, 2):
                        items.append(lambda j=j: s_pair(ci, j))
                    start_diag = nd
                else:
                    start_diag = 0
                for j in range(start_diag, n_jt):
                    items.append(lambda j=j: s_single(ci, j))
                return items

            # ---- global emission ----
            # chunk items with splices: AV of previous chunk early, proj of
            # next chunk's qk + this chunk's v late (hide proj latency).
            # Processing order pulls the first 512-chunk before (256,256).
            # Background-work queue: next chunk's projection steps, this
            # chunk's v projections, and the previous chunk's AV passes are
            # dribbled out between S items so the PE FIFO never carries a
            # long burst that would starve ACT of score blocks.
            proj_qk(0, 256)
            for ci in range(len(CHUNKS)):
                qs, qw = CHUNKS[ci]
                items = chunk_items(ci)
                n = len(items)
                bg = []
                if ci + 1 < len(CHUNKS):
                    nqs, nqw = CHUNKS[ci + 1]
                    bg += proj_qk_steps(nqs, nqs + nqw, 2 if nqw <= 256 else 4)
                for jj in range(qs // 128, (qs + qw) // 128):
                    bg.append(lambda j=jj: proj_v(j))
                if ci > 0:
                    for k in range(CHUNKS[ci - 1][1] // 128):
                        bg.append(lambda k=k: av_chunk(ci - 1, k))
                for idx, it in enumerate(items):
                    it()
                    rem = n - 1 - idx
                    take = len(bg) if rem == 0 else -(-len(bg) // max(rem, 1))
                    for _ in range(min(take, len(bg))):
                        bg.pop(0)()
            av_chunk(len(CHUNKS) - 1)

    nc.finalize()
    return nc


_NC_CACHE = None
LAST_RESULTS = None


def kernel(x, Wq, Wk, Wv, trace=False, **run_kwargs):
    global _NC_CACHE, LAST_RESULTS
    import ml_dtypes

    xdt = ml_dtypes.bfloat16
    x = np.asarray(x, dtype=np.float32)
    wqk = np.concatenate(
        [np.asarray(Wq, np.float32), np.asarray(Wk, np.float32)], axis=1
    ).astype(xdt)
    wqk_p = np.ascontiguousarray(wqk.reshape(CT, 128, 128).swapaxes(0, 1))
    wv_p = np.ascontiguousarray(
        np.asarray(Wv, np.float32).astype(xdt).reshape(CT, 128, D).swapaxes(0, 1)
    )

    if _NC_CACHE is None:
        _NC_CACHE = build_nc()
    nc = _NC_CACHE

    in_maps = []
    for b in range(N_CORES):
        xT = x[b].T.astype(xdt)  # [C, T]
        # [C, T] -> [p, t-tile, ct, 128]
        xT_p = np.ascontiguousarray(
            xT.reshape(CT, 128, NT, 128).transpose(1, 2, 0, 3)
        )
        in_maps.append({"xT": xT_p, "wqk": wqk_p, "wv": wv_p})

    res = run_bass_kernel_spmd(
        nc, in_maps, core_ids=list(range(N_CORES)), trace=trace, **run_kwargs
    )
    LAST_RESULTS = res
    out = np.stack(
        [
            np.asarray(res.results[b]["y"])
            .swapaxes(0, 1)
            .reshape(T, D)
            .astype(np.float32)
            for b in range(N_CORES)
        ],
        axis=0,
    )
    return out


if __name__ == "__main__":
    rng = np.random.default_rng(0)
    x = rng.standard_normal((B, T, C), dtype=np.float32)
    s = 1.0 / np.sqrt(C)
    Wq = rng.standard_normal((C, D), dtype=np.float32) * s
    Wk = rng.standard_normal((C, D), dtype=np.float32) * s
    Wv = rng.standard_normal((C, D), dtype=np.float32) * s
    out = kernel(x, Wq, Wk, Wv)
    print("out", out.shape, out.dtype, float(np.abs(out).max()))


# revision 58
# speedup vs baseline: 1.0561x; 1.0443x over previous
"""Single-head causal self-attention on 8 Trainium2 NeuronCores.

Problem: x[8, 2048, 1024], Wq/Wk/Wv[1024, 64] ->
  out[b] = softmax(causal((x[b]@Wq) @ (x[b]@Wk)^T / 8)) @ (x[b]@Wv)

Sharding: data-parallel over batch B=8, one batch element per core; weights
replicated. All device traffic is bf16 (PSUM accumulation stays fp32).

Per-core scheme:
  - [q^T;k^T] = Wqk^T @ x^T   (PE; x^T streamed per t-chunk)
  - V[t-tile, 64] natural layout directly: x^T-tile stationary, Wv moving
    (64-col matmuls) -> no V transpose; col 64 of V_aug = ones
  - S^T[j-tile, q-chunk] = (k^T tile)^T @ q^T, causal blocks only; blocks
    are bank-packed into [128,1024] PSUM tiles (blocks sharing a 2KB bank
    form one accumulation group: start pending-zeroes the whole bank, so
    later blocks write into zeroed bytes) and a single exp instruction
    covers the whole tile -- ACT per-instruction overhead is ~185ns, so
    packing 38 exps down to ~26 takes ~2.5us off the critical path
  - P^T = exp(S^T / 8)  (ACT, PSUM->SBUF bf16; no max-subtraction: scaled
    scores are ~N(0,1) so exp is well-conditioned)
  - diagonal 128x128 sub-blocks multiplied by a 0/1 triangle mask (DVE)
  - AV q-major: per q-tile, accumulate sum_j P^T-slice.T @ V_aug[j]
    (65-col matmuls, natural output; ones column = softmax denominator),
    then normalize rows by reciprocal of col 64 (DVE) straight from PSUM

Scheduling: ACT's ~22us of exp work is the critical path.  The emission
order forms one global pipeline: next-chunk projection matmuls, this
chunk's v-projections and the previous chunk's AV passes are dribbled
out between score blocks (a background-work queue) so the PE FIFO never
carries a burst that would starve ACT; the first chunks and x DMA
segments are narrow so exp starts as early as possible; dummy warmup
matmuls ramp the PE clock during the initial DMA wait.
"""

import numpy as np

import concourse.mybir as mybir
import concourse.tile as tile
from concourse import bacc
from concourse.bass_utils import run_bass_kernel_spmd
from concourse.masks import make_upper_triangular

N_CORES = 8
B, T, C, D = 8, 2048, 1024, 64
CT = C // 128          # 8 contraction tiles
NT = T // 128          # 16 row tiles
SCALE = float(1.0 / np.sqrt(D))

FP = mybir.dt.float32
BF = mybir.dt.bfloat16
XDT = mybir.dt.bfloat16  # fp8 fails the 2e-2 gate (scores + v for low-n_eff rows)

CHUNKS = [(0, 256), (256, 256), (512, 256), (768, 256), (1024, 512), (1536, 512)]
XSEGS = [(512, 768), (768, 1024), (1024, 1280), (1280, 1536),
         (1536, 1792), (1792, 2048)]  # after x0,wqk,x1; first seg before wv
WARMUP = 26


def build_nc():
    nc = bacc.Bacc("TRN2", target_bir_lowering=False)
    # x^T in t-tile-major layout [p, t-tile, ct, 128] so chunk DMAs are
    # contiguous per partition (>=1KB descriptors, no sub-512B penalty)
    xT_h = nc.dram_tensor("xT", [128, NT, CT, 128], XDT, kind="ExternalInput")
    wqk_h = nc.dram_tensor("wqk", [128, CT, 128], XDT, kind="ExternalInput")
    wv_h = nc.dram_tensor("wv", [128, CT, D], XDT, kind="ExternalInput")
    y_h = nc.dram_tensor("y", [128, NT, D], BF, kind="ExternalOutput")

    with tile.TileContext(nc) as tc:
        with (
            tc.tile_pool(name="const", bufs=1) as const,
            tc.tile_pool(name="pt", bufs=20) as ptp,
            tc.tile_pool(name="sc", bufs=8) as scp,
            tc.tile_pool(name="ps_w", bufs=2, space="PSUM") as ps_w,
            tc.tile_pool(name="ps_s", bufs=2, space="PSUM") as ps_s,
            tc.tile_pool(name="ps_o", bufs=2, space="PSUM") as ps_o,
        ):
            # ---- constants ----
            tri = const.tile([128, 128], BF, tag="tri")  # tri[p,f]=1.0 iff f>=p
            make_upper_triangular(nc, tri, val=1.0, diag=True)

            wqk_sb = const.tile([128, CT, 128], XDT, tag="wqk")
            wv_sb = const.tile([128, CT, D], XDT, tag="wv")
            xT_sb = const.tile([128, NT, CT, 128], XDT, tag="xT")
            # fine x segments up front (early projections), coarse later;
            # weights interleaved so x(0:256) lands first, wv by first AV
            def xseg(a, b):
                nc.sync.dma_start(
                    out=xT_sb[:, a // 128 : b // 128, :, :],
                    in_=xT_h[:, a // 128 : b // 128, :, :],
                )
            xseg(0, 256)
            nc.sync.dma_start(out=wqk_sb, in_=wqk_h[:, :, :])
            xseg(256, 512)
            xseg(*XSEGS[0])
            nc.sync.dma_start(out=wv_sb, in_=wv_h[:, :, :])
            for a, b in XSEGS[1:]:
                xseg(a, b)

            # PE clock warmup: dummy matmuls while the first DMAs land (the
            # p-state ramp needs ~3us of continuous PE activity)
            if WARMUP:
                wu = ps_w.tile([128, 512], FP, tag="w", name="wu")
                for i in range(WARMUP):
                    nc.tensor.matmul(
                        wu[:, 0:128], tri, tri, start=True, stop=True,
                        skip_group_check=True,
                    )

            qT = const.tile([64, T], BF, tag="qT")
            kT = const.tile([64, T], BF, tag="kT")
            V = const.tile([128, NT, D + 1], BF, tag="V")  # col D = ones
            nc.gpsimd.memset(V[:, :, D], 1.0)
            out_sb = const.tile([128, NT, D], BF, tag="out")

            # ---- emission helpers ----
            def proj_qk_steps(a, b, nsteps=4):
                """Emit the qk projection for t-cols [a,b) as `nsteps` thunks
                so the 8 accumulation matmuls can interleave with S blocks
                in the PE FIFO instead of forming a 1.7us monolith."""
                w = b - a
                state = {}

                def step(c0, c1, last):
                    if not state:
                        state["t"] = ps_w.tile(
                            [128, 512], FP, tag="w", name=f"pqk_{a}"
                        )
                    p_qk = state["t"]
                    for ct in range(c0, c1):
                        nc.tensor.matmul(
                            p_qk[:, 0:w],
                            wqk_sb[:, ct, :],
                            xT_sb[:, a // 128 : b // 128, ct, :],
                            start=(ct == 0),
                            stop=(ct == CT - 1),
                        )
                    if last:
                        if a < 512:  # ACT is idle this early; halve the chain
                            nc.scalar.copy(qT[:, a:b], p_qk[0:64, 0:w])
                        else:
                            nc.vector.tensor_copy(qT[:, a:b], p_qk[0:64, 0:w])
                        nc.vector.tensor_copy(kT[:, a:b], p_qk[64:128, 0:w])

                bounds = [CT * i // nsteps for i in range(nsteps + 1)]
                return [
                    (lambda c0=bounds[i], c1=bounds[i + 1], last=(i == nsteps - 1):
                     step(c0, c1, last))
                    for i in range(nsteps)
                ]

            def proj_qk(a, b):
                for f in proj_qk_steps(a, b, 1):
                    f()

            def proj_v(jt):
                p_v = ps_w.tile([128, 512], FP, tag="w", name=f"pv_{jt}")
                for ct in range(CT):
                    nc.tensor.matmul(
                        p_v[:, 0:D],
                        xT_sb[:, jt, ct, :],
                        wv_sb[:, ct, :],
                        start=(ct == 0),
                        stop=(ct == CT - 1),
                    )
                nc.vector.tensor_copy(V[:, jt, 0:D], p_v[:, 0:D])  # GPSIMD can't read PSUM

            # pt_view[(ci, jt)] = (tile, col offset) for AV consumption
            pt_view = {}

            def s_multi(ci, placed):
                """One [128,1024] score tile holding several causal blocks
                (jt, lo, off, w), bank-packed: blocks sharing a bank form one
                accumulation group (start marks the whole 2KB zero region, so
                later blocks in the bank write into pending-zero bytes).  A
                single exp covers the full span."""
                qs, qw = CHUNKS[ci]
                p_s = ps_s.tile(
                    [128, 1024], FP, tag="s", name=f"s_{ci}_{placed[0][0]}"
                )
                banks = {}
                for i, (jt, lo, off, w) in enumerate(placed):
                    banks.setdefault(off // 512, []).append(i)
                for i, (jt, lo, off, w) in enumerate(placed):
                    idxs = banks[off // 512]
                    nc.tensor.matmul(
                        p_s[:, off : off + w],
                        kT[:, jt * 128 : (jt + 1) * 128],
                        qT[:, qs + lo : qs + qw],
                        start=(i == idxs[0]),
                        stop=(i == idxs[-1]),
                        skip_group_check=True,
                    )
                span = placed[-1][2] + placed[-1][3]
                pt = ptp.tile(
                    [128, 1024], BF, tag="pt", name=f"pt_{ci}_{placed[0][0]}"
                )
                nc.scalar.activation(
                    pt[:, 0:span],
                    p_s[:, 0:span],
                    mybir.ActivationFunctionType.Exp,
                    scale=SCALE,
                )
                for jt, lo, off, w in placed:
                    if jt * 128 - qs >= 0:  # diagonal block: causal mask
                        nc.vector.tensor_mul(
                            pt[:, off : off + 128], pt[:, off : off + 128], tri
                        )
                    pt_view[(ci, jt)] = (pt, off - lo)

            def av_pass(ci, qi):
                qs, qw = CHUNKS[ci]
                qt = qs // 128 + qi
                ot = ps_o.tile([128, D + 1], FP, tag="o", name=f"o_{qt}")
                for jt in range(qt + 1):
                    pt, off = pt_view[(ci, jt)]
                    lo = max(jt * 128 - qs, 0)
                    assert qi * 128 >= lo
                    nc.tensor.matmul(
                        ot,
                        pt[:, off + qi * 128 : off + (qi + 1) * 128],
                        V[:, jt, :],
                        start=(jt == 0),
                        stop=(jt == qt),
                    )
                s2 = scp.tile([128, 2], FP, tag="s2", name=f"s2_{qt}")
                nc.vector.tensor_copy(s2[:, 0:1], ot[:, D : D + 1])
                nc.vector.reciprocal(s2[:, 1:2], s2[:, 0:1])
                nc.vector.tensor_scalar_mul(
                    out_sb[:, qt, :], ot[:, 0:D], s2[:, 1:2]
                )

            def av_chunk(ci, qi=None):
                qs, qw = CHUNKS[ci]
                rng = range(qw // 128) if qi is None else [qi]
                for q in rng:
                    av_pass(ci, q)
                if qi is not None and qi != qw // 128 - 1:
                    return
                hi = (qs + qw) // 128
                if hi % 4 == 0:
                    nc.sync.dma_start(
                        out=y_h[:, hi - 4 : hi, :], in_=out_sb[:, hi - 4 : hi, :]
                    )

            def chunk_items(ci):
                qs, qw = CHUNKS[ci]
                n_jt = (qs + qw) // 128
                blocks = [(jt, max(jt * 128 - qs, 0)) for jt in range(n_jt)]
                # last chunk: keep diagonal blocks unpacked so the final AV
                # passes (the schedule tail) see fine-grained exp completions
                solo_diag = ci == len(CHUNKS) - 1
                tiles = []
                cur, off = [], 0
                for jt, lo in blocks:
                    w = qw - lo
                    diag = jt * 128 - qs >= 0
                    if (off + w > 1024 or (off % 512 and off % 512 + w > 512)
                            or (solo_diag and diag and cur)):
                        tiles.append(cur)
                        cur, off = [], 0
                    cur.append((jt, lo, off, w))
                    off += w
                if cur:
                    tiles.append(cur)
                return [lambda p=p: s_multi(ci, p) for p in tiles]

            # ---- global emission ----
            # chunk items with splices: AV of previous chunk early, proj of
            # next chunk's qk + this chunk's v late (hide proj latency).
            # Processing order pulls the first 512-chunk before (256,256).
            # Background-work queue: next chunk's projection steps, this
            # chunk's v projections, and the previous chunk's AV passes are
            # dribbled out between S items so the PE FIFO never carries a
            # long burst that would starve ACT of score blocks.
            proj_qk(0, 256)
            for ci in range(len(CHUNKS)):
                qs, qw = CHUNKS[ci]
                items = chunk_items(ci)
                n = len(items)
                bg = []
                if ci + 1 < len(CHUNKS):
                    nqs, nqw = CHUNKS[ci + 1]
                    bg += proj_qk_steps(nqs, nqs + nqw, 2 if nqw <= 256 else 4)
                for jj in range(qs // 128, (qs + qw) // 128):
                    bg.append(lambda j=jj: proj_v(j))
                if ci > 0:
                    for k in range(CHUNKS[ci - 1][1] // 128):
                        bg.append(lambda k=k: av_chunk(ci - 1, k))
                for idx, it in enumerate(items):
                    it()
                    rem = n - 1 - idx
                    take = len(bg) if rem == 0 else -(-len(bg) // max(rem, 1))
                    for _ in range(min(take, len(bg))):
                        bg.pop(0)()
            av_chunk(len(CHUNKS) - 1)

    nc.finalize()
    return nc


_NC_CACHE = None
LAST_RESULTS = None


def kernel(x, Wq, Wk, Wv, trace=False, **run_kwargs):
    global _NC_CACHE, LAST_RESULTS
    import ml_dtypes

    xdt = ml_dtypes.bfloat16
    x = np.asarray(x, dtype=np.float32)
    wqk = np.concatenate(
        [np.asarray(Wq, np.float32), np.asarray(Wk, np.float32)], axis=1
    ).astype(xdt)
    wqk_p = np.ascontiguousarray(wqk.reshape(CT, 128, 128).swapaxes(0, 1))
    wv_p = np.ascontiguousarray(
        np.asarray(Wv, np.float32).astype(xdt).reshape(CT, 128, D).swapaxes(0, 1)
    )

    if _NC_CACHE is None:
        _NC_CACHE = build_nc()
    nc = _NC_CACHE

    in_maps = []
    for b in range(N_CORES):
        xT = x[b].T.astype(xdt)  # [C, T]
        # [C, T] -> [p, t-tile, ct, 128]
        xT_p = np.ascontiguousarray(
            xT.reshape(CT, 128, NT, 128).transpose(1, 2, 0, 3)
        )
        in_maps.append({"xT": xT_p, "wqk": wqk_p, "wv": wv_p})

    res = run_bass_kernel_spmd(
        nc, in_maps, core_ids=list(range(N_CORES)), trace=trace, **run_kwargs
    )
    LAST_RESULTS = res
    out = np.stack(
        [
            np.asarray(res.results[b]["y"])
            .swapaxes(0, 1)
            .reshape(T, D)
            .astype(np.float32)
            for b in range(N_CORES)
        ],
        axis=0,
    )
    return out


if __name__ == "__main__":
    rng = np.random.default_rng(0)
    x = rng.standard_normal((B, T, C), dtype=np.float32)
    s = 1.0 / np.sqrt(C)
    Wq = rng.standard_normal((C, D), dtype=np.float32) * s
    Wk = rng.standard_normal((C, D), dtype=np.float32) * s
    Wv = rng.standard_normal((C, D), dtype=np.float32) * s
    out = kernel(x, Wq, Wk, Wv)
    print("out", out.shape, out.dtype, float(np.abs(out).max()))


# revision 63
# speedup vs baseline: 1.0863x; 1.0286x over previous
"""Single-head causal self-attention on 8 Trainium2 NeuronCores.

Problem: x[8, 2048, 1024], Wq/Wk/Wv[1024, 64] ->
  out[b] = softmax(causal((x[b]@Wq) @ (x[b]@Wk)^T / 8)) @ (x[b]@Wv)

Sharding: data-parallel over batch B=8, one batch element per core; weights
replicated. All device traffic is bf16 (PSUM accumulation stays fp32).

Per-core scheme:
  - [q^T;k^T] = Wqk^T @ x^T   (PE; x^T streamed per t-chunk)
  - V[t-tile, 64] natural layout directly: x^T-tile stationary, Wv moving
    (64-col matmuls) -> no V transpose; col 64 of V_aug = ones
  - S^T[j-tile, q-chunk] = (k^T tile)^T @ q^T, causal blocks only; blocks
    are bank-packed into [128,1024] PSUM tiles (blocks sharing a 2KB bank
    form one accumulation group: start pending-zeroes the whole bank, so
    later blocks write into zeroed bytes) and a single exp instruction
    covers the whole tile -- ACT per-instruction overhead is ~185ns, so
    packing 38 exps down to ~26 takes ~2.5us off the critical path
  - P^T = exp(S^T / 8)  (ACT, PSUM->SBUF bf16; no max-subtraction: scaled
    scores are ~N(0,1) so exp is well-conditioned)
  - diagonal 128x128 sub-blocks multiplied by a 0/1 triangle mask (DVE)
  - AV q-major: per q-tile, accumulate sum_j P^T-slice.T @ V_aug[j]
    (65-col matmuls, natural output; ones column = softmax denominator),
    then normalize rows by reciprocal of col 64 (DVE) straight from PSUM

Scheduling: ACT's ~22us of exp work is the critical path.  The emission
order forms one global pipeline: next-chunk projection matmuls, this
chunk's v-projections and the previous chunk's AV passes are dribbled
out between score blocks (a background-work queue) so the PE FIFO never
carries a burst that would starve ACT; the first chunks and x DMA
segments are narrow so exp starts as early as possible; dummy warmup
matmuls ramp the PE clock during the initial DMA wait.
"""

import numpy as np

import concourse.mybir as mybir
import concourse.tile as tile
from concourse import bacc
from concourse.bass_utils import run_bass_kernel_spmd
from concourse.masks import make_upper_triangular

N_CORES = 8
B, T, C, D = 8, 2048, 1024, 64
CT = C // 128          # 8 contraction tiles
NT = T // 128          # 16 row tiles
SCALE = float(1.0 / np.sqrt(D))

FP = mybir.dt.float32
BF = mybir.dt.bfloat16
XDT = mybir.dt.bfloat16  # fp8 fails the 2e-2 gate (scores + v for low-n_eff rows)

CHUNKS = [(0, 256), (256, 256), (512, 256), (768, 256), (1024, 256),
          (1280, 256), (1536, 256), (1792, 256)]
XSEGS = [(512, 768), (768, 1024), (1024, 1280), (1280, 1536),
         (1536, 1792), (1792, 2048)]  # after x0,wqk,x1; first seg before wv
WARMUP = 26


def build_nc():
    nc = bacc.Bacc("TRN2", target_bir_lowering=False)
    # x^T in t-tile-major layout [p, t-tile, ct, 128] so chunk DMAs are
    # contiguous per partition (>=1KB descriptors, no sub-512B penalty)
    xT_h = nc.dram_tensor("xT", [128, NT, CT, 128], XDT, kind="ExternalInput")
    wqk_h = nc.dram_tensor("wqk", [128, CT, 128], XDT, kind="ExternalInput")
    wv_h = nc.dram_tensor("wv", [128, CT, D], XDT, kind="ExternalInput")
    y_h = nc.dram_tensor("y", [128, NT, D], BF, kind="ExternalOutput")

    with tile.TileContext(nc) as tc:
        with (
            tc.tile_pool(name="const", bufs=1) as const,
            tc.tile_pool(name="pt", bufs=20) as ptp,
            tc.tile_pool(name="sc", bufs=8) as scp,
            tc.tile_pool(name="ps_w", bufs=2, space="PSUM") as ps_w,
            tc.tile_pool(name="ps_s", bufs=2, space="PSUM") as ps_s,
            tc.tile_pool(name="ps_o", bufs=2, space="PSUM") as ps_o,
        ):
            # ---- constants ----
            tri = const.tile([128, 128], BF, tag="tri")  # tri[p,f]=1.0 iff f>=p
            make_upper_triangular(nc, tri, val=1.0, diag=True)

            wqk_sb = const.tile([128, CT, 128], XDT, tag="wqk")
            wv_sb = const.tile([128, CT, D], XDT, tag="wv")
            xT_sb = const.tile([128, NT, CT, 128], XDT, tag="xT")
            # fine x segments up front (early projections), coarse later;
            # weights interleaved so x(0:256) lands first, wv by first AV
            def xseg(a, b):
                nc.sync.dma_start(
                    out=xT_sb[:, a // 128 : b // 128, :, :],
                    in_=xT_h[:, a // 128 : b // 128, :, :],
                )
            xseg(0, 256)
            nc.sync.dma_start(out=wqk_sb, in_=wqk_h[:, :, :])
            xseg(256, 512)
            xseg(*XSEGS[0])
            nc.sync.dma_start(out=wv_sb, in_=wv_h[:, :, :])
            for a, b in XSEGS[1:]:
                xseg(a, b)

            # PE clock warmup: dummy matmuls while the first DMAs land (the
            # p-state ramp needs ~3us of continuous PE activity)
            if WARMUP:
                wu = ps_w.tile([128, 512], FP, tag="w", name="wu")
                for i in range(WARMUP):
                    nc.tensor.matmul(
                        wu[:, 0:128], tri, tri, start=True, stop=True,
                        skip_group_check=True,
                    )

            qT = const.tile([64, T], BF, tag="qT")
            kT = const.tile([64, T], BF, tag="kT")
            V = const.tile([128, NT, D + 1], BF, tag="V")  # col D = ones
            nc.gpsimd.memset(V[:, :, D], 1.0)
            out_sb = const.tile([128, NT, D], BF, tag="out")

            # ---- emission helpers ----
            def proj_qk_steps(a, b, nsteps=4):
                """Emit the qk projection for t-cols [a,b) as `nsteps` thunks
                so the 8 accumulation matmuls can interleave with S blocks
                in the PE FIFO instead of forming a 1.7us monolith."""
                w = b - a
                state = {}

                def step(c0, c1, last):
                    if not state:
                        state["t"] = ps_w.tile(
                            [128, 512], FP, tag="w", name=f"pqk_{a}"
                        )
                    p_qk = state["t"]
                    for ct in range(c0, c1):
                        nc.tensor.matmul(
                            p_qk[:, 0:w],
                            wqk_sb[:, ct, :],
                            xT_sb[:, a // 128 : b // 128, ct, :],
                            start=(ct == 0),
                            stop=(ct == CT - 1),
                        )
                    if last:
                        if a < 512:  # ACT is idle this early; halve the chain
                            nc.scalar.copy(qT[:, a:b], p_qk[0:64, 0:w])
                        else:
                            nc.vector.tensor_copy(qT[:, a:b], p_qk[0:64, 0:w])
                        nc.vector.tensor_copy(kT[:, a:b], p_qk[64:128, 0:w])

                bounds = [CT * i // nsteps for i in range(nsteps + 1)]
                return [
                    (lambda c0=bounds[i], c1=bounds[i + 1], last=(i == nsteps - 1):
                     step(c0, c1, last))
                    for i in range(nsteps)
                ]

            def proj_qk(a, b):
                for f in proj_qk_steps(a, b, 1):
                    f()

            def proj_v(jt):
                p_v = ps_w.tile([128, 512], FP, tag="w", name=f"pv_{jt}")
                for ct in range(CT):
                    nc.tensor.matmul(
                        p_v[:, 0:D],
                        xT_sb[:, jt, ct, :],
                        wv_sb[:, ct, :],
                        start=(ct == 0),
                        stop=(ct == CT - 1),
                    )
                nc.vector.tensor_copy(V[:, jt, 0:D], p_v[:, 0:D])  # GPSIMD can't read PSUM

            # pt_view[(ci, jt)] = (tile, col offset) for AV consumption
            pt_view = {}

            def s_multi(ci, placed):
                """One [128,1024] score tile holding several causal blocks
                (jt, lo, off, w), bank-packed: blocks sharing a bank form one
                accumulation group (start marks the whole 2KB zero region, so
                later blocks in the bank write into pending-zero bytes).  A
                single exp covers the full span."""
                qs, qw = CHUNKS[ci]
                p_s = ps_s.tile(
                    [128, 1024], FP, tag="s", name=f"s_{ci}_{placed[0][0]}"
                )
                banks = {}
                for i, (jt, lo, off, w) in enumerate(placed):
                    banks.setdefault(off // 512, []).append(i)
                for i, (jt, lo, off, w) in enumerate(placed):
                    idxs = banks[off // 512]
                    nc.tensor.matmul(
                        p_s[:, off : off + w],
                        kT[:, jt * 128 : (jt + 1) * 128],
                        qT[:, qs + lo : qs + qw],
                        start=(i == idxs[0]),
                        stop=(i == idxs[-1]),
                        skip_group_check=True,
                    )
                span = placed[-1][2] + placed[-1][3]
                pt = ptp.tile(
                    [128, 1024], BF, tag="pt", name=f"pt_{ci}_{placed[0][0]}"
                )
                nc.scalar.activation(
                    pt[:, 0:span],
                    p_s[:, 0:span],
                    mybir.ActivationFunctionType.Exp,
                    scale=SCALE,
                )
                for jt, lo, off, w in placed:
                    if jt * 128 - qs >= 0:  # diagonal block: causal mask
                        # on GPSIMD (SBUF-only op): keeps the DVE FIFO free
                        # for the critical qT/kT evacuation copies
                        nc.gpsimd.tensor_mul(
                            pt[:, off : off + 128], pt[:, off : off + 128], tri
                        )
                    pt_view[(ci, jt)] = (pt, off - lo)

            def av_pass(ci, qi):
                qs, qw = CHUNKS[ci]
                qt = qs // 128 + qi
                ot = ps_o.tile([128, D + 1], FP, tag="o", name=f"o_{qt}")
                for jt in range(qt + 1):
                    pt, off = pt_view[(ci, jt)]
                    lo = max(jt * 128 - qs, 0)
                    assert qi * 128 >= lo
                    nc.tensor.matmul(
                        ot,
                        pt[:, off + qi * 128 : off + (qi + 1) * 128],
                        V[:, jt, :],
                        start=(jt == 0),
                        stop=(jt == qt),
                    )
                s2 = scp.tile([128, 2], FP, tag="s2", name=f"s2_{qt}")
                nc.vector.tensor_copy(s2[:, 0:1], ot[:, D : D + 1])
                nc.vector.reciprocal(s2[:, 1:2], s2[:, 0:1])
                nc.vector.tensor_scalar_mul(
                    out_sb[:, qt, :], ot[:, 0:D], s2[:, 1:2]
                )

            def av_chunk(ci, qi=None):
                qs, qw = CHUNKS[ci]
                rng = range(qw // 128) if qi is None else [qi]
                for q in rng:
                    av_pass(ci, q)
                if qi is not None and qi != qw // 128 - 1:
                    return
                hi = (qs + qw) // 128
                if hi % 4 == 0:
                    nc.sync.dma_start(
                        out=y_h[:, hi - 4 : hi, :], in_=out_sb[:, hi - 4 : hi, :]
                    )

            def chunk_items(ci):
                qs, qw = CHUNKS[ci]
                n_jt = (qs + qw) // 128
                blocks = [(jt, max(jt * 128 - qs, 0)) for jt in range(n_jt)]
                # last chunk: keep diagonal blocks unpacked so the final AV
                # passes (the schedule tail) see fine-grained exp completions
                solo_diag = ci == len(CHUNKS) - 1
                tiles = []
                cur, off = [], 0
                for jt, lo in blocks:
                    w = qw - lo
                    diag = jt * 128 - qs >= 0
                    if (off + w > 1024 or (off % 512 and off % 512 + w > 512)
                            or (solo_diag and diag and cur)):
                        tiles.append(cur)
                        cur, off = [], 0
                    cur.append((jt, lo, off, w))
                    off += w
                if cur:
                    tiles.append(cur)
                return [lambda p=p: s_multi(ci, p) for p in tiles]

            # ---- global emission ----
            # chunk items with splices: AV of previous chunk early, proj of
            # next chunk's qk + this chunk's v late (hide proj latency).
            # Processing order pulls the first 512-chunk before (256,256).
            # Background-work queue: next chunk's projection steps, this
            # chunk's v projections, and the previous chunk's AV passes are
            # dribbled out between S items so the PE FIFO never carries a
            # long burst that would starve ACT of score blocks.
            proj_qk(0, 256)
            for ci in range(len(CHUNKS)):
                qs, qw = CHUNKS[ci]
                items = chunk_items(ci)
                n = len(items)
                bg = []
                if ci + 1 < len(CHUNKS):
                    nqs, nqw = CHUNKS[ci + 1]
                    bg += proj_qk_steps(nqs, nqs + nqw, 2 if nqw <= 256 else 4)
                for jj in range(qs // 128, (qs + qw) // 128):
                    bg.append(lambda j=jj: proj_v(j))
                if ci > 0:
                    for k in range(CHUNKS[ci - 1][1] // 128):
                        bg.append(lambda k=k: av_chunk(ci - 1, k))
                for idx, it in enumerate(items):
                    it()
                    rem = n - 1 - idx
                    take = len(bg) if rem == 0 else -(-len(bg) // max(rem, 1))
                    for _ in range(min(take, len(bg))):
                        bg.pop(0)()
            av_chunk(len(CHUNKS) - 1)

    nc.finalize()
    return nc


_NC_CACHE = None
LAST_RESULTS = None


def kernel(x, Wq, Wk, Wv, trace=False, **run_kwargs):
    global _NC_CACHE, LAST_RESULTS
    import ml_dtypes

    xdt = ml_dtypes.bfloat16
    x = np.asarray(x, dtype=np.float32)
    wqk = np.concatenate(
        [np.asarray(Wq, np.float32), np.asarray(Wk, np.float32)], axis=1
    ).astype(xdt)
    wqk_p = np.ascontiguousarray(wqk.reshape(CT, 128, 128).swapaxes(0, 1))
    wv_p = np.ascontiguousarray(
        np.asarray(Wv, np.float32).astype(xdt).reshape(CT, 128, D).swapaxes(0, 1)
    )

    if _NC_CACHE is None:
        _NC_CACHE = build_nc()
    nc = _NC_CACHE

    in_maps = []
    for b in range(N_CORES):
        xT = x[b].T.astype(xdt)  # [C, T]
        # [C, T] -> [p, t-tile, ct, 128]
        xT_p = np.ascontiguousarray(
            xT.reshape(CT, 128, NT, 128).transpose(1, 2, 0, 3)
        )
        in_maps.append({"xT": xT_p, "wqk": wqk_p, "wv": wv_p})

    res = run_bass_kernel_spmd(
        nc, in_maps, core_ids=list(range(N_CORES)), trace=trace, **run_kwargs
    )
    LAST_RESULTS = res
    out = np.stack(
        [
            np.asarray(res.results[b]["y"])
            .swapaxes(0, 1)
            .reshape(T, D)
            .astype(np.float32)
            for b in range(N_CORES)
        ],
        axis=0,
    )
    return out


if __name__ == "__main__":
    rng = np.random.default_rng(0)
    x = rng.standard_normal((B, T, C), dtype=np.float32)
    s = 1.0 / np.sqrt(C)
    Wq = rng.standard_normal((C, D), dtype=np.float32) * s
    Wk = rng.standard_normal((C, D), dtype=np.float32) * s
    Wv = rng.standard_normal((C, D), dtype=np.float32) * s
    out = kernel(x, Wq, Wk, Wv)
    print("out", out.shape, out.dtype, float(np.abs(out).max()))


# revision 69
# speedup vs baseline: 1.0874x; 1.0010x over previous
"""Single-head causal self-attention on 8 Trainium2 NeuronCores.

Problem: x[8, 2048, 1024], Wq/Wk/Wv[1024, 64] ->
  out[b] = softmax(causal((x[b]@Wq) @ (x[b]@Wk)^T / 8)) @ (x[b]@Wv)

Sharding: data-parallel over batch B=8, one batch element per core; weights
replicated. All device traffic is bf16 (PSUM accumulation stays fp32).

Per-core scheme:
  - [q^T;k^T] = Wqk^T @ x^T   (PE; x^T streamed per t-chunk)
  - V[t-tile, 64] natural layout directly: x^T-tile stationary, Wv moving
    (64-col matmuls) -> no V transpose; col 64 of V_aug = ones
  - S^T[j-tile, q-chunk] = (k^T tile)^T @ q^T, causal blocks only; blocks
    are bank-packed into [128,1024] PSUM tiles (blocks sharing a 2KB bank
    form one accumulation group: start pending-zeroes the whole bank, so
    later blocks write into zeroed bytes) and a single exp instruction
    covers the whole tile -- ACT per-instruction overhead is ~185ns, so
    packing 38 exps down to ~26 takes ~2.5us off the critical path
  - P^T = exp(S^T / 8)  (ACT, PSUM->SBUF bf16; no max-subtraction: scaled
    scores are ~N(0,1) so exp is well-conditioned)
  - diagonal 128x128 sub-blocks multiplied by a 0/1 triangle mask (DVE)
  - AV q-major: per q-tile, accumulate sum_j P^T-slice.T @ V_aug[j]
    (65-col matmuls, natural output; ones column = softmax denominator),
    then normalize rows by reciprocal of col 64 (DVE) straight from PSUM

Scheduling: ACT's ~22us of exp work is the critical path.  The emission
order forms one global pipeline: next-chunk projection matmuls, this
chunk's v-projections and the previous chunk's AV passes are dribbled
out between score blocks (a background-work queue) so the PE FIFO never
carries a burst that would starve ACT; the first chunks and x DMA
segments are narrow so exp starts as early as possible; dummy warmup
matmuls ramp the PE clock during the initial DMA wait.
"""

import numpy as np

import concourse.mybir as mybir
import concourse.tile as tile
from concourse import bacc
from concourse.bass_utils import run_bass_kernel_spmd
from concourse.masks import make_upper_triangular

N_CORES = 8
B, T, C, D = 8, 2048, 1024, 64
CT = C // 128          # 8 contraction tiles
NT = T // 128          # 16 row tiles
SCALE = float(1.0 / np.sqrt(D))

FP = mybir.dt.float32
BF = mybir.dt.bfloat16
XDT = mybir.dt.bfloat16  # fp8 fails the 2e-2 gate (scores + v for low-n_eff rows)

CHUNKS = [(0, 256), (256, 256), (512, 256), (768, 256), (1024, 256),
          (1280, 256), (1536, 256), (1792, 256)]
XSEGS = [(512, 768), (768, 1024), (1024, 1280), (1280, 1536),
         (1536, 1792), (1792, 2048)]  # after x0,wqk,x1; first seg before wv
WARMUP = 26


def build_nc():
    nc = bacc.Bacc("TRN2", target_bir_lowering=False)
    # x^T in t-tile-major layout [p, t-tile, ct, 128] so chunk DMAs are
    # contiguous per partition (>=1KB descriptors, no sub-512B penalty)
    xT_h = nc.dram_tensor("xT", [128, NT, CT, 128], XDT, kind="ExternalInput")
    wqk_h = nc.dram_tensor("wqk", [128, CT, 128], XDT, kind="ExternalInput")
    wv_h = nc.dram_tensor("wv", [128, CT, D], XDT, kind="ExternalInput")
    y_h = nc.dram_tensor("y", [128, NT, D], BF, kind="ExternalOutput")

    with tile.TileContext(nc) as tc:
        with (
            tc.tile_pool(name="const", bufs=1) as const,
            tc.tile_pool(name="pt", bufs=20) as ptp,
            tc.tile_pool(name="sc", bufs=8) as scp,
            tc.tile_pool(name="ps_w", bufs=2, space="PSUM") as ps_w,
            tc.tile_pool(name="ps_s", bufs=2, space="PSUM") as ps_s,
            tc.tile_pool(name="ps_o", bufs=2, space="PSUM") as ps_o,
        ):
            # ---- constants ----
            tri = const.tile([128, 128], BF, tag="tri")  # tri[p,f]=1.0 iff f>=p
            make_upper_triangular(nc, tri, val=1.0, diag=True)

            wqk_sb = const.tile([128, CT, 128], XDT, tag="wqk")
            wv_sb = const.tile([128, CT, D], XDT, tag="wv")
            xT_sb = const.tile([128, NT, CT, 128], XDT, tag="xT")
            # fine x segments up front (early projections), coarse later;
            # weights interleaved so x(0:256) lands first, wv by first AV
            def xseg(a, b):
                nc.sync.dma_start(
                    out=xT_sb[:, a // 128 : b // 128, :, :],
                    in_=xT_h[:, a // 128 : b // 128, :, :],
                )
            xseg(0, 256)
            nc.sync.dma_start(out=wqk_sb, in_=wqk_h[:, :, :])
            xseg(256, 512)
            xseg(*XSEGS[0])
            nc.sync.dma_start(out=wv_sb, in_=wv_h[:, :, :])
            for a, b in XSEGS[1:]:
                xseg(a, b)

            # PE clock warmup: dummy matmuls while the first DMAs land (the
            # p-state ramp needs ~3us of continuous PE activity)
            if WARMUP:
                wu = ps_w.tile([128, 512], FP, tag="w", name="wu")
                for i in range(WARMUP):
                    nc.tensor.matmul(
                        wu[:, 0:128], tri, tri, start=True, stop=True,
                        skip_group_check=True,
                    )

            qT = const.tile([64, T], BF, tag="qT")
            kT = const.tile([64, T], BF, tag="kT")
            V = const.tile([128, NT, D + 1], BF, tag="V")  # col D = ones
            nc.gpsimd.memset(V[:, :, D], 1.0)
            out_sb = const.tile([128, NT, D], BF, tag="out")

            # ---- emission helpers ----
            def proj_qk_steps(a, b, nsteps=4):
                """Emit the qk projection for t-cols [a,b) as `nsteps` thunks
                so the 8 accumulation matmuls can interleave with S blocks
                in the PE FIFO instead of forming a 1.7us monolith."""
                w = b - a
                state = {}

                def step(c0, c1, last):
                    if not state:
                        state["t"] = ps_w.tile(
                            [128, 512], FP, tag="w", name=f"pqk_{a}"
                        )
                    p_qk = state["t"]
                    for ct in range(c0, c1):
                        nc.tensor.matmul(
                            p_qk[:, 0:w],
                            wqk_sb[:, ct, :],
                            xT_sb[:, a // 128 : b // 128, ct, :],
                            start=(ct == 0),
                            stop=(ct == CT - 1),
                        )
                    if last:
                        if a < 512:  # ACT is idle this early; halve the chain
                            nc.scalar.copy(qT[:, a:b], p_qk[0:64, 0:w])
                        else:
                            nc.vector.tensor_copy(qT[:, a:b], p_qk[0:64, 0:w])
                        nc.vector.tensor_copy(kT[:, a:b], p_qk[64:128, 0:w])

                bounds = [CT * i // nsteps for i in range(nsteps + 1)]
                return [
                    (lambda c0=bounds[i], c1=bounds[i + 1], last=(i == nsteps - 1):
                     step(c0, c1, last))
                    for i in range(nsteps)
                ]

            def proj_qk(a, b):
                for f in proj_qk_steps(a, b, 1):
                    f()

            def proj_v(jt):
                p_v = ps_w.tile([128, 512], FP, tag="w", name=f"pv_{jt}")
                for ct in range(CT):
                    nc.tensor.matmul(
                        p_v[:, 0:D],
                        xT_sb[:, jt, ct, :],
                        wv_sb[:, ct, :],
                        start=(ct == 0),
                        stop=(ct == CT - 1),
                    )
                nc.vector.tensor_copy(V[:, jt, 0:D], p_v[:, 0:D])  # GPSIMD can't read PSUM

            # pt_view[(ci, jt)] = (tile, col offset) for AV consumption
            pt_view = {}

            def s_multi(ci, placed):
                """One [128,1024] score tile holding several causal blocks
                (jt, lo, off, w), bank-packed: blocks sharing a bank form one
                accumulation group (start marks the whole 2KB zero region, so
                later blocks in the bank write into pending-zero bytes).  A
                single exp covers the full span."""
                qs, qw = CHUNKS[ci]
                p_s = ps_s.tile(
                    [128, 1024], FP, tag="s", name=f"s_{ci}_{placed[0][0]}"
                )
                banks = {}
                for i, (jt, lo, off, w) in enumerate(placed):
                    banks.setdefault(off // 512, []).append(i)
                for i, (jt, lo, off, w) in enumerate(placed):
                    idxs = banks[off // 512]
                    nc.tensor.matmul(
                        p_s[:, off : off + w],
                        kT[:, jt * 128 : (jt + 1) * 128],
                        qT[:, qs + lo : qs + qw],
                        start=(i == idxs[0]),
                        stop=(i == idxs[-1]),
                        skip_group_check=True,
                    )
                span = placed[-1][2] + placed[-1][3]
                pt = ptp.tile(
                    [128, 1024], BF, tag="pt", name=f"pt_{ci}_{placed[0][0]}"
                )
                nc.scalar.activation(
                    pt[:, 0:span],
                    p_s[:, 0:span],
                    mybir.ActivationFunctionType.Exp,
                    scale=SCALE,
                )
                for jt, lo, off, w in placed:
                    if jt * 128 - qs >= 0:  # diagonal block: causal mask
                        # on GPSIMD (SBUF-only op): keeps the DVE FIFO free
                        # for the critical qT/kT evacuation copies
                        nc.gpsimd.tensor_mul(
                            pt[:, off : off + 128], pt[:, off : off + 128], tri
                        )
                    pt_view[(ci, jt)] = (pt, off - lo)

            def av_pass(ci, qi):
                qs, qw = CHUNKS[ci]
                qt = qs // 128 + qi
                ot = ps_o.tile([128, D + 1], FP, tag="o", name=f"o_{qt}")
                for jt in range(qt + 1):
                    pt, off = pt_view[(ci, jt)]
                    lo = max(jt * 128 - qs, 0)
                    assert qi * 128 >= lo
                    nc.tensor.matmul(
                        ot,
                        pt[:, off + qi * 128 : off + (qi + 1) * 128],
                        V[:, jt, :],
                        start=(jt == 0),
                        stop=(jt == qt),
                    )
                s2 = scp.tile([128, 2], FP, tag="s2", name=f"s2_{qt}")
                nc.vector.reciprocal(s2[:, 1:2], ot[:, D : D + 1])
                nc.vector.tensor_scalar_mul(
                    out_sb[:, qt, :], ot[:, 0:D], s2[:, 1:2]
                )

            def av_chunk(ci, qi=None):
                qs, qw = CHUNKS[ci]
                rng = range(qw // 128) if qi is None else [qi]
                for q in rng:
                    av_pass(ci, q)
                if qi is not None and qi != qw // 128 - 1:
                    return
                hi = (qs + qw) // 128
                if hi % 4 == 0:
                    nc.sync.dma_start(
                        out=y_h[:, hi - 4 : hi, :], in_=out_sb[:, hi - 4 : hi, :]
                    )

            def chunk_items(ci):
                qs, qw = CHUNKS[ci]
                n_jt = (qs + qw) // 128
                blocks = [(jt, max(jt * 128 - qs, 0)) for jt in range(n_jt)]
                # last chunk: keep diagonal blocks unpacked so the final AV
                # passes (the schedule tail) see fine-grained exp completions
                solo_diag = ci == len(CHUNKS) - 1
                tiles = []
                cur, off = [], 0
                for jt, lo in blocks:
                    w = qw - lo
                    diag = jt * 128 - qs >= 0
                    if (off + w > 1024 or (off % 512 and off % 512 + w > 512)
                            or (solo_diag and diag and cur)):
                        tiles.append(cur)
                        cur, off = [], 0
                    cur.append((jt, lo, off, w))
                    off += w
                if cur:
                    tiles.append(cur)
                return [lambda p=p: s_multi(ci, p) for p in tiles]

            # ---- global emission ----
            # chunk items with splices: AV of previous chunk early, proj of
            # next chunk's qk + this chunk's v late (hide proj latency).
            # Processing order pulls the first 512-chunk before (256,256).
            # Background-work queue: next chunk's projection steps, this
            # chunk's v projections, and the previous chunk's AV passes are
            # dribbled out between S items so the PE FIFO never carries a
            # long burst that would starve ACT of score blocks.
            proj_qk(0, 256)
            for ci in range(len(CHUNKS)):
                qs, qw = CHUNKS[ci]
                items = chunk_items(ci)
                n = len(items)
                bg = []
                if ci + 1 < len(CHUNKS):
                    nqs, nqw = CHUNKS[ci + 1]
                    bg += proj_qk_steps(nqs, nqs + nqw, 2 if nqw <= 256 else 4)
                for jj in range(qs // 128, (qs + qw) // 128):
                    bg.append(lambda j=jj: proj_v(j))
                if ci > 0:
                    for k in range(CHUNKS[ci - 1][1] // 128):
                        bg.append(lambda k=k: av_chunk(ci - 1, k))
                for idx, it in enumerate(items):
                    it()
                    rem = n - 1 - idx
                    take = len(bg) if rem == 0 else -(-len(bg) // max(rem, 1))
                    for _ in range(min(take, len(bg))):
                        bg.pop(0)()
            av_chunk(len(CHUNKS) - 1)

    nc.finalize()
    return nc


_NC_CACHE = None
LAST_RESULTS = None


def kernel(x, Wq, Wk, Wv, trace=False, **run_kwargs):
    global _NC_CACHE, LAST_RESULTS
    import ml_dtypes

    xdt = ml_dtypes.bfloat16
    x = np.asarray(x, dtype=np.float32)
    wqk = np.concatenate(
        [np.asarray(Wq, np.float32), np.asarray(Wk, np.float32)], axis=1
    ).astype(xdt)
    wqk_p = np.ascontiguousarray(wqk.reshape(CT, 128, 128).swapaxes(0, 1))
    wv_p = np.ascontiguousarray(
        np.asarray(Wv, np.float32).astype(xdt).reshape(CT, 128, D).swapaxes(0, 1)
    )

    if _NC_CACHE is None:
        _NC_CACHE = build_nc()
    nc = _NC_CACHE

    in_maps = []
    for b in range(N_CORES):
        xT = x[b].T.astype(xdt)  # [C, T]
        # [C, T] -> [p, t-tile, ct, 128]
        xT_p = np.ascontiguousarray(
            xT.reshape(CT, 128, NT, 128).transpose(1, 2, 0, 3)
        )
        in_maps.append({"xT": xT_p, "wqk": wqk_p, "wv": wv_p})

    res = run_bass_kernel_spmd(
        nc, in_maps, core_ids=list(range(N_CORES)), trace=trace, **run_kwargs
    )
    LAST_RESULTS = res
    out = np.stack(
        [
            np.asarray(res.results[b]["y"])
            .swapaxes(0, 1)
            .reshape(T, D)
            .astype(np.float32)
            for b in range(N_CORES)
        ],
        axis=0,
    )
    return out


if __name__ == "__main__":
    rng = np.random.default_rng(0)
    x = rng.standard_normal((B, T, C), dtype=np.float32)
    s = 1.0 / np.sqrt(C)
    Wq = rng.standard_normal((C, D), dtype=np.float32) * s
    Wk = rng.standard_normal((C, D), dtype=np.float32) * s
    Wv = rng.standard_normal((C, D), dtype=np.float32) * s
    out = kernel(x, Wq, Wk, Wv)
    print("out", out.shape, out.dtype, float(np.abs(out).max()))


# revision 72
# speedup vs baseline: 1.0875x; 1.0001x over previous
"""Single-head causal self-attention on 8 Trainium2 NeuronCores.

Problem: x[8, 2048, 1024], Wq/Wk/Wv[1024, 64] ->
  out[b] = softmax(causal((x[b]@Wq) @ (x[b]@Wk)^T / 8)) @ (x[b]@Wv)

Sharding: data-parallel over batch B=8, one batch element per core; weights
replicated. All device traffic is bf16 (PSUM accumulation stays fp32).

Per-core scheme:
  - [q^T;k^T] = Wqk^T @ x^T   (PE; x^T streamed per t-chunk)
  - V[t-tile, 64] natural layout directly: x^T-tile stationary, Wv moving
    (64-col matmuls) -> no V transpose; col 64 of V_aug = ones
  - S^T[j-tile, q-chunk] = (k^T tile)^T @ q^T, causal blocks only; blocks
    are bank-packed into [128,1024] PSUM tiles (blocks sharing a 2KB bank
    form one accumulation group: start pending-zeroes the whole bank, so
    later blocks write into zeroed bytes) and a single exp instruction
    covers the whole tile -- ACT per-instruction overhead is ~185ns, so
    packing 38 exps down to ~26 takes ~2.5us off the critical path
  - P^T = exp(S^T / 8)  (ACT, PSUM->SBUF bf16; no max-subtraction: scaled
    scores are ~N(0,1) so exp is well-conditioned)
  - diagonal 128x128 sub-blocks multiplied by a 0/1 triangle mask (DVE)
  - AV q-major: per q-tile, accumulate sum_j P^T-slice.T @ V_aug[j]
    (65-col matmuls, natural output; ones column = softmax denominator),
    then normalize rows by reciprocal of col 64 (DVE) straight from PSUM

Scheduling: ACT's ~22us of exp work is the critical path.  The emission
order forms one global pipeline: next-chunk projection matmuls, this
chunk's v-projections and the previous chunk's AV passes are dribbled
out between score blocks (a background-work queue) so the PE FIFO never
carries a burst that would starve ACT; the first chunks and x DMA
segments are narrow so exp starts as early as possible; dummy warmup
matmuls ramp the PE clock during the initial DMA wait.
"""

import numpy as np

import concourse.mybir as mybir
import concourse.tile as tile
from concourse import bacc
from concourse.bass_utils import run_bass_kernel_spmd
from concourse.masks import make_upper_triangular

N_CORES = 8
B, T, C, D = 8, 2048, 1024, 64
CT = C // 128          # 8 contraction tiles
NT = T // 128          # 16 row tiles
SCALE = float(1.0 / np.sqrt(D))

FP = mybir.dt.float32
BF = mybir.dt.bfloat16
XDT = mybir.dt.bfloat16  # fp8 fails the 2e-2 gate (scores + v for low-n_eff rows)

CHUNKS = [(0, 256), (256, 256), (512, 256), (768, 256), (1024, 256),
          (1280, 256), (1536, 256), (1792, 256)]
XSEGS = [(512, 768), (768, 1024), (1024, 1280), (1280, 1536),
         (1536, 1792), (1792, 2048)]  # after x0,wqk,x1; first seg before wv
WARMUP = 26


def build_nc():
    nc = bacc.Bacc("TRN2", target_bir_lowering=False)
    # x^T in t-tile-major layout [p, t-tile, ct, 128] so chunk DMAs are
    # contiguous per partition (>=1KB descriptors, no sub-512B penalty)
    xT_h = nc.dram_tensor("xT", [128, NT, CT, 128], XDT, kind="ExternalInput")
    wqk_h = nc.dram_tensor("wqk", [128, CT, 128], XDT, kind="ExternalInput")
    wv_h = nc.dram_tensor("wv", [128, CT, D], XDT, kind="ExternalInput")
    y_h = nc.dram_tensor("y", [128, NT, D], BF, kind="ExternalOutput")

    with tile.TileContext(nc) as tc:
        with (
            tc.tile_pool(name="const", bufs=1) as const,
            tc.tile_pool(name="pt", bufs=20) as ptp,
            tc.tile_pool(name="sc", bufs=8) as scp,
            tc.tile_pool(name="ps_w", bufs=2, space="PSUM") as ps_w,
            tc.tile_pool(name="ps_s", bufs=2, space="PSUM") as ps_s,
            tc.tile_pool(name="ps_o", bufs=2, space="PSUM") as ps_o,
        ):
            # ---- constants ----
            tri = const.tile([128, 128], BF, tag="tri")  # tri[p,f]=1.0 iff f>=p
            make_upper_triangular(nc, tri, val=1.0, diag=True)

            wqk_sb = const.tile([128, CT, 128], XDT, tag="wqk")
            wv_sb = const.tile([128, CT, D], XDT, tag="wv")
            xT_sb = const.tile([128, NT, CT, 128], XDT, tag="xT")
            # fine x segments up front (early projections), coarse later;
            # weights interleaved so x(0:256) lands first, wv by first AV
            def xseg(a, b):
                nc.sync.dma_start(
                    out=xT_sb[:, a // 128 : b // 128, :, :],
                    in_=xT_h[:, a // 128 : b // 128, :, :],
                )
            xseg(0, 128)
            nc.sync.dma_start(out=wqk_sb, in_=wqk_h[:, :, :])
            xseg(128, 256)
            xseg(256, 512)
            xseg(*XSEGS[0])
            nc.sync.dma_start(out=wv_sb, in_=wv_h[:, :, :])
            for a, b in XSEGS[1:]:
                xseg(a, b)

            # PE clock warmup: dummy matmuls while the first DMAs land (the
            # p-state ramp needs ~3us of continuous PE activity)
            if WARMUP:
                wu = ps_w.tile([128, 512], FP, tag="w", name="wu")
                for i in range(WARMUP):
                    nc.tensor.matmul(
                        wu[:, 0:128], tri, tri, start=True, stop=True,
                        skip_group_check=True,
                    )

            qT = const.tile([64, T], BF, tag="qT")
            kT = const.tile([64, T], BF, tag="kT")
            V = const.tile([128, NT, D + 1], BF, tag="V")  # col D = ones
            nc.gpsimd.memset(V[:, :, D], 1.0)
            out_sb = const.tile([128, NT, D], BF, tag="out")

            # ---- emission helpers ----
            def proj_qk_steps(a, b, nsteps=4):
                """Emit the qk projection for t-cols [a,b) as `nsteps` thunks
                so the 8 accumulation matmuls can interleave with S blocks
                in the PE FIFO instead of forming a 1.7us monolith."""
                w = b - a
                state = {}

                def step(c0, c1, last):
                    if not state:
                        state["t"] = ps_w.tile(
                            [128, 512], FP, tag="w", name=f"pqk_{a}"
                        )
                    p_qk = state["t"]
                    for ct in range(c0, c1):
                        nc.tensor.matmul(
                            p_qk[:, 0:w],
                            wqk_sb[:, ct, :],
                            xT_sb[:, a // 128 : b // 128, ct, :],
                            start=(ct == 0),
                            stop=(ct == CT - 1),
                        )
                    if last:
                        if a < 512:  # ACT is idle this early; halve the chain
                            nc.scalar.copy(qT[:, a:b], p_qk[0:64, 0:w])
                        else:
                            nc.vector.tensor_copy(qT[:, a:b], p_qk[0:64, 0:w])
                        nc.vector.tensor_copy(kT[:, a:b], p_qk[64:128, 0:w])

                bounds = [CT * i // nsteps for i in range(nsteps + 1)]
                return [
                    (lambda c0=bounds[i], c1=bounds[i + 1], last=(i == nsteps - 1):
                     step(c0, c1, last))
                    for i in range(nsteps)
                ]

            def proj_qk(a, b):
                for f in proj_qk_steps(a, b, 1):
                    f()

            def proj_v(jt):
                p_v = ps_w.tile([128, 512], FP, tag="w", name=f"pv_{jt}")
                for ct in range(CT):
                    nc.tensor.matmul(
                        p_v[:, 0:D],
                        xT_sb[:, jt, ct, :],
                        wv_sb[:, ct, :],
                        start=(ct == 0),
                        stop=(ct == CT - 1),
                    )
                nc.vector.tensor_copy(V[:, jt, 0:D], p_v[:, 0:D])  # GPSIMD can't read PSUM

            # pt_view[(ci, jt)] = (tile, col offset) for AV consumption
            pt_view = {}

            def s_multi(ci, placed):
                """One [128,1024] score tile holding several causal blocks
                (jt, lo, off, w), bank-packed: blocks sharing a bank form one
                accumulation group (start marks the whole 2KB zero region, so
                later blocks in the bank write into pending-zero bytes).  A
                single exp covers the full span."""
                qs, qw = CHUNKS[ci]
                p_s = ps_s.tile(
                    [128, 1024], FP, tag="s", name=f"s_{ci}_{placed[0][0]}"
                )
                banks = {}
                for i, (jt, lo, off, w) in enumerate(placed):
                    banks.setdefault(off // 512, []).append(i)
                for i, (jt, lo, off, w) in enumerate(placed):
                    idxs = banks[off // 512]
                    nc.tensor.matmul(
                        p_s[:, off : off + w],
                        kT[:, jt * 128 : (jt + 1) * 128],
                        qT[:, qs + lo : qs + qw],
                        start=(i == idxs[0]),
                        stop=(i == idxs[-1]),
                        skip_group_check=True,
                    )
                span = placed[-1][2] + placed[-1][3]
                pt = ptp.tile(
                    [128, 1024], BF, tag="pt", name=f"pt_{ci}_{placed[0][0]}"
                )
                nc.scalar.activation(
                    pt[:, 0:span],
                    p_s[:, 0:span],
                    mybir.ActivationFunctionType.Exp,
                    scale=SCALE,
                )
                for jt, lo, off, w in placed:
                    if jt * 128 - qs >= 0:  # diagonal block: causal mask
                        # on GPSIMD (SBUF-only op): keeps the DVE FIFO free
                        # for the critical qT/kT evacuation copies
                        nc.gpsimd.tensor_mul(
                            pt[:, off : off + 128], pt[:, off : off + 128], tri
                        )
                    pt_view[(ci, jt)] = (pt, off - lo)

            def av_pass(ci, qi):
                qs, qw = CHUNKS[ci]
                qt = qs // 128 + qi
                ot = ps_o.tile([128, D + 1], FP, tag="o", name=f"o_{qt}")
                for jt in range(qt + 1):
                    pt, off = pt_view[(ci, jt)]
                    lo = max(jt * 128 - qs, 0)
                    assert qi * 128 >= lo
                    nc.tensor.matmul(
                        ot,
                        pt[:, off + qi * 128 : off + (qi + 1) * 128],
                        V[:, jt, :],
                        start=(jt == 0),
                        stop=(jt == qt),
                    )
                s2 = scp.tile([128, 2], FP, tag="s2", name=f"s2_{qt}")
                nc.vector.reciprocal(s2[:, 1:2], ot[:, D : D + 1])
                nc.vector.tensor_scalar_mul(
                    out_sb[:, qt, :], ot[:, 0:D], s2[:, 1:2]
                )

            def av_chunk(ci, qi=None):
                qs, qw = CHUNKS[ci]
                rng = range(qw // 128) if qi is None else [qi]
                for q in rng:
                    av_pass(ci, q)
                if qi is not None and qi != qw // 128 - 1:
                    return
                hi = (qs + qw) // 128
                if hi % 4 == 0:
                    nc.sync.dma_start(
                        out=y_h[:, hi - 4 : hi, :], in_=out_sb[:, hi - 4 : hi, :]
                    )

            def chunk_items(ci):
                qs, qw = CHUNKS[ci]
                n_jt = (qs + qw) // 128
                blocks = [(jt, max(jt * 128 - qs, 0)) for jt in range(n_jt)]
                # last chunk: keep diagonal blocks unpacked so the final AV
                # passes (the schedule tail) see fine-grained exp completions
                solo_diag = ci == len(CHUNKS) - 1
                tiles = []
                cur, off = [], 0
                for jt, lo in blocks:
                    w = qw - lo
                    diag = jt * 128 - qs >= 0
                    if (off + w > 1024 or (off % 512 and off % 512 + w > 512)
                            or (solo_diag and diag and cur)):
                        tiles.append(cur)
                        cur, off = [], 0
                    cur.append((jt, lo, off, w))
                    off += w
                if cur:
                    tiles.append(cur)
                return [lambda p=p: s_multi(ci, p) for p in tiles]

            # ---- global emission ----
            # chunk items with splices: AV of previous chunk early, proj of
            # next chunk's qk + this chunk's v late (hide proj latency).
            # Processing order pulls the first 512-chunk before (256,256).
            # Background-work queue: next chunk's projection steps, this
            # chunk's v projections, and the previous chunk's AV passes are
            # dribbled out between S items so the PE FIFO never carries a
            # long burst that would starve ACT of score blocks.
            proj_qk(0, 128)
            proj_qk(128, 256)
            for ci in range(len(CHUNKS)):
                qs, qw = CHUNKS[ci]
                items = chunk_items(ci)
                n = len(items)
                bg = []
                if ci + 1 < len(CHUNKS):
                    nqs, nqw = CHUNKS[ci + 1]
                    bg += proj_qk_steps(nqs, nqs + nqw, 2 if nqw <= 256 else 4)
                for jj in range(qs // 128, (qs + qw) // 128):
                    bg.append(lambda j=jj: proj_v(j))
                if ci > 0:
                    for k in range(CHUNKS[ci - 1][1] // 128):
                        bg.append(lambda k=k: av_chunk(ci - 1, k))
                for idx, it in enumerate(items):
                    it()
                    rem = n - 1 - idx
                    take = len(bg) if rem == 0 else -(-len(bg) // max(rem, 1))
                    for _ in range(min(take, len(bg))):
                        bg.pop(0)()
            av_chunk(len(CHUNKS) - 1)

    nc.finalize()
    return nc


_NC_CACHE = None
LAST_RESULTS = None


def kernel(x, Wq, Wk, Wv, trace=False, **run_kwargs):
    global _NC_CACHE, LAST_RESULTS
    import ml_dtypes

    xdt = ml_dtypes.bfloat16
    x = np.asarray(x, dtype=np.float32)
    wqk = np.concatenate(
        [np.asarray(Wq, np.float32), np.asarray(Wk, np.float32)], axis=1
    ).astype(xdt)
    wqk_p = np.ascontiguousarray(wqk.reshape(CT, 128, 128).swapaxes(0, 1))
    wv_p = np.ascontiguousarray(
        np.asarray(Wv, np.float32).astype(xdt).reshape(CT, 128, D).swapaxes(0, 1)
    )

    if _NC_CACHE is None:
        _NC_CACHE = build_nc()
    nc = _NC_CACHE

    in_maps = []
    for b in range(N_CORES):
        xT = x[b].T.astype(xdt)  # [C, T]
        # [C, T] -> [p, t-tile, ct, 128]
        xT_p = np.ascontiguousarray(
            xT.reshape(CT, 128, NT, 128).transpose(1, 2, 0, 3)
        )
        in_maps.append({"xT": xT_p, "wqk": wqk_p, "wv": wv_p})

    res = run_bass_kernel_spmd(
        nc, in_maps, core_ids=list(range(N_CORES)), trace=trace, **run_kwargs
    )
    LAST_RESULTS = res
    out = np.stack(
        [
            np.asarray(res.results[b]["y"])
            .swapaxes(0, 1)
            .reshape(T, D)
            .astype(np.float32)
            for b in range(N_CORES)
        ],
        axis=0,
    )
    return out


if __name__ == "__main__":
    rng = np.random.default_rng(0)
    x = rng.standard_normal((B, T, C), dtype=np.float32)
    s = 1.0 / np.sqrt(C)
    Wq = rng.standard_normal((C, D), dtype=np.float32) * s
    Wk = rng.standard_normal((C, D), dtype=np.float32) * s
    Wv = rng.standard_normal((C, D), dtype=np.float32) * s
    out = kernel(x, Wq, Wk, Wv)
    print("out", out.shape, out.dtype, float(np.abs(out).max()))


# revision 79
# speedup vs baseline: 1.0891x; 1.0014x over previous
"""Single-head causal self-attention on 8 Trainium2 NeuronCores.

Problem: x[8, 2048, 1024], Wq/Wk/Wv[1024, 64] ->
  out[b] = softmax(causal((x[b]@Wq) @ (x[b]@Wk)^T / 8)) @ (x[b]@Wv)

Sharding: data-parallel over batch B=8, one batch element per core; weights
replicated. All device traffic is bf16 (PSUM accumulation stays fp32).

Per-core scheme:
  - [q^T;k^T] = Wqk^T @ x^T   (PE; x^T streamed per t-chunk)
  - V[t-tile, 64] natural layout directly: x^T-tile stationary, Wv moving
    (64-col matmuls) -> no V transpose; col 64 of V_aug = ones
  - S^T[j-tile, q-chunk] = (k^T tile)^T @ q^T, causal blocks only; blocks
    are bank-packed into [128,1024] PSUM tiles (blocks sharing a 2KB bank
    form one accumulation group: start pending-zeroes the whole bank, so
    later blocks write into zeroed bytes) and a single exp instruction
    covers the whole tile -- ACT per-instruction overhead is ~185ns, so
    packing (4 blocks/tile at 256-wide chunks) cuts the exp instruction
    count to ~22 and takes ~3us off the critical path
  - P^T = exp(S^T / 8)  (ACT, PSUM->SBUF bf16; no max-subtraction: scaled
    scores are ~N(0,1) so exp is well-conditioned)
  - diagonal 128x128 sub-blocks multiplied by a 0/1 triangle mask (DVE)
  - AV q-major: per q-tile, accumulate sum_j P^T-slice.T @ V_aug[j]
    (65-col matmuls, natural output; ones column = softmax denominator),
    then normalize rows by reciprocal of col 64 (DVE) straight from PSUM

Scheduling: ACT's ~22us of exp work is the critical path.  The emission
order forms one global pipeline: next-chunk projection matmuls, this
chunk's v-projections and the previous chunk's AV passes are dribbled
out between score blocks (a background-work queue) so the PE FIFO never
carries a burst that would starve ACT; the first chunks and x DMA
segments are narrow so exp starts as early as possible; dummy warmup
matmuls ramp the PE clock during the initial DMA wait.
"""

import numpy as np

import concourse.mybir as mybir
import concourse.tile as tile
from concourse import bacc
from concourse.bass_utils import run_bass_kernel_spmd
from concourse.masks import make_upper_triangular

N_CORES = 8
B, T, C, D = 8, 2048, 1024, 64
CT = C // 128          # 8 contraction tiles
NT = T // 128          # 16 row tiles
SCALE = float(1.0 / np.sqrt(D))

FP = mybir.dt.float32
BF = mybir.dt.bfloat16
XDT = mybir.dt.bfloat16  # fp8 fails the 2e-2 gate (scores + v for low-n_eff rows)

CHUNKS = [(0, 256), (256, 256), (512, 256), (768, 256), (1024, 256),
          (1280, 256), (1536, 256), (1792, 256)]
XSEGS = [(512, 768), (768, 1024), (1024, 1280), (1280, 1536),
         (1536, 1792), (1792, 2048)]  # after x0,wqk,x1; first seg before wv
WARMUP = 26


def build_nc():
    nc = bacc.Bacc("TRN2", target_bir_lowering=False)
    # x^T in t-tile-major layout [p, t-tile, ct, 128] so chunk DMAs are
    # contiguous per partition (>=1KB descriptors, no sub-512B penalty)
    xT_h = nc.dram_tensor("xT", [128, NT, CT, 128], XDT, kind="ExternalInput")
    wqk_h = nc.dram_tensor("wqk", [128, CT, 128], XDT, kind="ExternalInput")
    wv_h = nc.dram_tensor("wv", [128, CT, D], XDT, kind="ExternalInput")
    y_h = nc.dram_tensor("y", [128, NT, D], BF, kind="ExternalOutput")

    with tile.TileContext(nc) as tc:
        with (
            tc.tile_pool(name="const", bufs=1) as const,
            tc.tile_pool(name="pt", bufs=20) as ptp,
            tc.tile_pool(name="sc", bufs=8) as scp,
            tc.tile_pool(name="ps_w", bufs=2, space="PSUM") as ps_w,
            tc.tile_pool(name="ps_s", bufs=2, space="PSUM") as ps_s,
            tc.tile_pool(name="ps_o", bufs=2, space="PSUM") as ps_o,
        ):
            # ---- constants ----
            tri = const.tile([128, 128], BF, tag="tri")  # tri[p,f]=1.0 iff f>=p
            make_upper_triangular(nc, tri, val=1.0, diag=True)

            wqk_sb = const.tile([128, CT, 128], XDT, tag="wqk")
            wv_sb = const.tile([128, CT, D], XDT, tag="wv")
            xT_sb = const.tile([128, NT, CT, 128], XDT, tag="xT")
            # fine x segments up front (early projections), coarse later;
            # weights interleaved so x(0:256) lands first, wv by first AV
            def xseg(a, b):
                nc.sync.dma_start(
                    out=xT_sb[:, a // 128 : b // 128, :, :],
                    in_=xT_h[:, a // 128 : b // 128, :, :],
                )
            xseg(0, 128)
            nc.sync.dma_start(out=wqk_sb, in_=wqk_h[:, :, :])
            xseg(128, 256)
            xseg(256, 512)
            xseg(*XSEGS[0])
            nc.sync.dma_start(out=wv_sb, in_=wv_h[:, :, :])
            for a, b in XSEGS[1:]:
                xseg(a, b)

            # PE clock warmup: dummy matmuls while the first DMAs land (the
            # p-state ramp needs ~3us of continuous PE activity)
            if WARMUP:
                wu = ps_w.tile([128, 512], FP, tag="w", name="wu")
                for i in range(WARMUP):
                    nc.tensor.matmul(
                        wu[:, 0:128], tri, tri, start=True, stop=True,
                        skip_group_check=True,
                    )

            qT = const.tile([64, T], BF, tag="qT")
            kT = const.tile([64, T], BF, tag="kT")
            V = const.tile([128, NT, D + 1], BF, tag="V")  # col D = ones
            nc.gpsimd.memset(V[:, :, D], 1.0)
            out_sb = const.tile([128, NT, D], BF, tag="out")

            # ---- emission helpers ----
            def proj_qk_steps(a, b, nsteps=4):
                """Emit the qk projection for t-cols [a,b) as `nsteps` thunks
                so the 8 accumulation matmuls can interleave with S blocks
                in the PE FIFO instead of forming a 1.7us monolith."""
                w = b - a
                state = {}

                def step(c0, c1, last):
                    if not state:
                        state["t"] = ps_w.tile(
                            [128, 512], FP, tag="w", name=f"pqk_{a}"
                        )
                    p_qk = state["t"]
                    for ct in range(c0, c1):
                        nc.tensor.matmul(
                            p_qk[:, 0:w],
                            wqk_sb[:, ct, :],
                            xT_sb[:, a // 128 : b // 128, ct, :],
                            start=(ct == 0),
                            stop=(ct == CT - 1),
                        )
                    if last:
                        if a < 512:  # ACT is idle this early; halve the chain
                            nc.scalar.copy(qT[:, a:b], p_qk[0:64, 0:w])
                        else:
                            nc.vector.tensor_copy(qT[:, a:b], p_qk[0:64, 0:w])
                        nc.vector.tensor_copy(kT[:, a:b], p_qk[64:128, 0:w])

                bounds = [CT * i // nsteps for i in range(nsteps + 1)]
                return [
                    (lambda c0=bounds[i], c1=bounds[i + 1], last=(i == nsteps - 1):
                     step(c0, c1, last))
                    for i in range(nsteps)
                ]

            def proj_qk(a, b):
                for f in proj_qk_steps(a, b, 1):
                    f()

            def proj_v(jt):
                p_v = ps_w.tile([128, 512], FP, tag="w", name=f"pv_{jt}")
                for ct in range(CT):
                    nc.tensor.matmul(
                        p_v[:, 0:D],
                        xT_sb[:, jt, ct, :],
                        wv_sb[:, ct, :],
                        start=(ct == 0),
                        stop=(ct == CT - 1),
                    )
                nc.vector.tensor_copy(V[:, jt, 0:D], p_v[:, 0:D])  # GPSIMD can't read PSUM

            # pt_view[(ci, jt)] = (tile, col offset) for AV consumption
            pt_view = {}

            def s_multi(ci, placed):
                """One [128,1024] score tile holding several causal blocks
                (jt, lo, off, w), bank-packed: blocks sharing a bank form one
                accumulation group (start marks the whole 2KB zero region, so
                later blocks in the bank write into pending-zero bytes).  A
                single exp covers the full span."""
                qs, qw = CHUNKS[ci]
                p_s = ps_s.tile(
                    [128, 1024], FP, tag="s", name=f"s_{ci}_{placed[0][0]}"
                )
                banks = {}
                for i, (jt, lo, off, w) in enumerate(placed):
                    banks.setdefault(off // 512, []).append(i)
                for i, (jt, lo, off, w) in enumerate(placed):
                    idxs = banks[off // 512]
                    nc.tensor.matmul(
                        p_s[:, off : off + w],
                        kT[:, jt * 128 : (jt + 1) * 128],
                        qT[:, qs + lo : qs + qw],
                        start=(i == idxs[0]),
                        stop=(i == idxs[-1]),
                        skip_group_check=True,
                    )
                span = placed[-1][2] + placed[-1][3]
                pt = ptp.tile(
                    [128, 1024], BF, tag="pt", name=f"pt_{ci}_{placed[0][0]}"
                )
                nc.scalar.activation(
                    pt[:, 0:span],
                    p_s[:, 0:span],
                    mybir.ActivationFunctionType.Exp,
                    scale=SCALE,
                )
                for jt, lo, off, w in placed:
                    if jt * 128 - qs >= 0:  # diagonal block: causal mask
                        # on GPSIMD (SBUF-only op): keeps the DVE FIFO free
                        # for the critical qT/kT evacuation copies
                        nc.gpsimd.tensor_mul(
                            pt[:, off : off + 128], pt[:, off : off + 128], tri
                        )
                    pt_view[(ci, jt)] = (pt, off - lo)

            def av_pass(ci, qi):
                qs, qw = CHUNKS[ci]
                qt = qs // 128 + qi
                ot = ps_o.tile([128, D + 1], FP, tag="o", name=f"o_{qt}")
                for jt in range(qt + 1):
                    pt, off = pt_view[(ci, jt)]
                    lo = max(jt * 128 - qs, 0)
                    assert qi * 128 >= lo
                    nc.tensor.matmul(
                        ot,
                        pt[:, off + qi * 128 : off + (qi + 1) * 128],
                        V[:, jt, :],
                        start=(jt == 0),
                        stop=(jt == qt),
                    )
                s2 = scp.tile([128, 2], FP, tag="s2", name=f"s2_{qt}")
                nc.vector.reciprocal(s2[:, 1:2], ot[:, D : D + 1])
                nc.vector.tensor_scalar_mul(
                    out_sb[:, qt, :], ot[:, 0:D], s2[:, 1:2]
                )

            def av_chunk(ci, qi=None):
                qs, qw = CHUNKS[ci]
                rng = range(qw // 128) if qi is None else [qi]
                for q in rng:
                    av_pass(ci, q)
                if qi is not None and qi != qw // 128 - 1:
                    return
                hi = (qs + qw) // 128
                if hi == NT - 2:
                    nc.sync.dma_start(
                        out=y_h[:, hi - 2 : hi, :], in_=out_sb[:, hi - 2 : hi, :]
                    )
                elif hi == NT:
                    nc.sync.dma_start(
                        out=y_h[:, hi - 2 : hi, :], in_=out_sb[:, hi - 2 : hi, :]
                    )
                elif hi % 4 == 0:
                    nc.sync.dma_start(
                        out=y_h[:, hi - 4 : hi, :], in_=out_sb[:, hi - 4 : hi, :]
                    )

            def chunk_items(ci):
                qs, qw = CHUNKS[ci]
                n_jt = (qs + qw) // 128
                blocks = [(jt, max(jt * 128 - qs, 0)) for jt in range(n_jt)]
                # last chunk: keep diagonal blocks unpacked so the final AV
                # passes (the schedule tail) see fine-grained exp completions
                solo_diag = ci == len(CHUNKS) - 1
                tiles = []
                cur, off = [], 0
                for jt, lo in blocks:
                    w = qw - lo
                    diag = jt * 128 - qs >= 0
                    if (off + w > 1024 or (off % 512 and off % 512 + w > 512)
                            or (solo_diag and diag and cur)):
                        tiles.append(cur)
                        cur, off = [], 0
                    cur.append((jt, lo, off, w))
                    off += w
                if cur:
                    tiles.append(cur)
                return [lambda p=p: s_multi(ci, p) for p in tiles]

            # ---- global emission ----
            # chunk items with splices: AV of previous chunk early, proj of
            # next chunk's qk + this chunk's v late (hide proj latency).
            # Processing order pulls the first 512-chunk before (256,256).
            # Background-work queue: next chunk's projection steps, this
            # chunk's v projections, and the previous chunk's AV passes are
            # dribbled out between S items so the PE FIFO never carries a
            # long burst that would starve ACT of score blocks.
            proj_qk(0, 128)
            proj_qk(128, 256)
            for ci in range(len(CHUNKS)):
                qs, qw = CHUNKS[ci]
                items = chunk_items(ci)
                n = len(items)
                bg = []
                if ci + 1 < len(CHUNKS):
                    nqs, nqw = CHUNKS[ci + 1]
                    bg += proj_qk_steps(nqs, nqs + nqw, 2 if nqw <= 256 else 4)
                for jj in range(qs // 128, (qs + qw) // 128):
                    bg.append(lambda j=jj: proj_v(j))
                if ci > 0:
                    for k in range(CHUNKS[ci - 1][1] // 128):
                        bg.append(lambda k=k: av_chunk(ci - 1, k))
                for idx, it in enumerate(items):
                    it()
                    rem = n - 1 - idx
                    take = len(bg) if rem == 0 else -(-len(bg) // max(rem, 1))
                    for _ in range(min(take, len(bg))):
                        bg.pop(0)()
            av_chunk(len(CHUNKS) - 1)

    nc.finalize()
    return nc


_NC_CACHE = None
LAST_RESULTS = None


def kernel(x, Wq, Wk, Wv, trace=False, **run_kwargs):
    global _NC_CACHE, LAST_RESULTS
    import ml_dtypes

    xdt = ml_dtypes.bfloat16
    x = np.asarray(x, dtype=np.float32)
    wqk = np.concatenate(
        [np.asarray(Wq, np.float32), np.asarray(Wk, np.float32)], axis=1
    ).astype(xdt)
    wqk_p = np.ascontiguousarray(wqk.reshape(CT, 128, 128).swapaxes(0, 1))
    wv_p = np.ascontiguousarray(
        np.asarray(Wv, np.float32).astype(xdt).reshape(CT, 128, D).swapaxes(0, 1)
    )

    if _NC_CACHE is None:
        _NC_CACHE = build_nc()
    nc = _NC_CACHE

    in_maps = []
    for b in range(N_CORES):
        xT = x[b].T.astype(xdt)  # [C, T]
        # [C, T] -> [p, t-tile, ct, 128]
        xT_p = np.ascontiguousarray(
            xT.reshape(CT, 128, NT, 128).transpose(1, 2, 0, 3)
        )
        in_maps.append({"xT": xT_p, "wqk": wqk_p, "wv": wv_p})

    res = run_bass_kernel_spmd(
        nc, in_maps, core_ids=list(range(N_CORES)), trace=trace, **run_kwargs
    )
    LAST_RESULTS = res
    out = np.stack(
        [
            np.asarray(res.results[b]["y"])
            .swapaxes(0, 1)
            .reshape(T, D)
            .astype(np.float32)
            for b in range(N_CORES)
        ],
        axis=0,
    )
    return out


if __name__ == "__main__":
    rng = np.random.default_rng(0)
    x = rng.standard_normal((B, T, C), dtype=np.float32)
    s = 1.0 / np.sqrt(C)
    Wq = rng.standard_normal((C, D), dtype=np.float32) * s
    Wk = rng.standard_normal((C, D), dtype=np.float32) * s
    Wv = rng.standard_normal((C, D), dtype=np.float32) * s
    out = kernel(x, Wq, Wk, Wv)
    print("out", out.shape, out.dtype, float(np.abs(out).max()))
